# revision 21
# baseline (speedup 1.0000x reference)
"""Trainium2 Bass kernel for the quantum-circuit KG-embedding scoring model.

score(s,p,o) = Re(<B_o h | W_p | B_s h>), B_e / W_p = 24-gate circuit blocks,
h = |+>^6.  State dim 64 complex = 128 reals [re(64) | im(64)].

Device algorithm (8 cores, SPMD), fp16 throughout the heavy paths:

  Chain phase (DVE, fp16 tile-minor layout [128 ent, 128 amp-real, T]):
    A-chain: evolve 1280 entities/core (10 tiles innermost) through the
    entity block (product-state doubling + 18 CRots).  Every DVE op has a
    stride-1 fp16 innermost dim -> 2x perf mode; coefficient tables are
    pre-replicated over the gate's low amp bits so no op exceeds 3 free
    dims (TENSOR3D) and no gate needs group splitting.
    Store T rows fp16 -> AllGather -> T_full [10240,128] fp16.
    W-chain: same machinery on 13 tiles = 26 relation slots x 64 basis
    columns, overlapping the AllGather.  Expand to W^T rows in DRAM.

  Phase C (supertiles of 512 elements, one relation each):
    dma_gather(transpose=True) pulls Ts^T / To^T [128 state, 13312 elems]
    straight from T_full (0.34ns/descriptor on gpsimd vs ~12ns for
    per-row indirect DMA; transfers spread over 16 DMA engines).
    dma_gather pulls per-supertile W^T [128,128] tiles.
    Per supertile: PE fp16 matmul Y = W @ Ts^T (psum f32), DVE
    prod = Y * To^T (fp16), PE ones-matmul column-sum -> scores.

Host does only: trig for the 200-relation tables, index sort/packing,
output unpermute (same division of labour as the reference baseline).
"""

import sys
import numpy as np

for _p in ("/opt/trn_rl_repo",):
    if _p not in sys.path:
        sys.path.insert(0, _p)

import concourse.bass as bass
import concourse.bacc as bacc
import concourse.mybir as mybir
from concourse import tile
from concourse.bass_utils import run_bass_kernel_spmd

F32 = mybir.dt.float32
F16 = mybir.dt.float16
I16 = mybir.dt.int16
ALU = mybir.AluOpType
ACTFN = mybir.ActivationFunctionType

P = 128
Q = 6
NA = 64                      # 2^Q amplitudes
NCORES = 8
E, R, B = 10000, 200, 65536
ETILES = 10                  # entity tiles per core
EPC = ETILES * P             # 1280 entities per core
EPAD = EPC * NCORES          # 10240 padded entity rows
WT = 13                      # W-chain tiles per core (2 rel slots each)
RSLOT = 2 * WT               # 26 relation slots per core
NST = 26                     # phase-C supertiles per core
STW = 512                    # supertile width (elements)
NIDX_T = NST * STW           # 13312 gathered T rows per table per core
NIDX_W = NST * P             # 3328 gathered W^T rows per core
R2 = float(2.0 ** -0.5)
PI = float(np.pi)

# CRot gate list: (control, target) wire pairs, in circuit order
CROTS = [(q, (q + off) % Q) for off in (1, 2, 3) for q in range(Q)]

# per-gate geometry: amp bit positions cpos=5-c (control), tpos=5-t
_GEO = []
for (c, t) in CROTS:
    cpos, tpos = 5 - c, 5 - t
    hi, lo = max(cpos, tpos), min(cpos, tpos)
    A = 1 << (5 - hi)
    Bm = 1 << (hi - lo - 1)
    C = 1 << lo
    _GEO.append((cpos, tpos, hi, lo, A, Bm, C))

# replicated coefficient table layout: per gate, 7 slots each replicated C
# times: block [7, C, T] at offset OFF[g] (in slot-columns of width T)
_OFF = []
_o = 0
for (_, _, _, _, _, _, C) in _GEO:
    _OFF.append(_o)
    _o += 7 * C
NREP = _o                    # total replicated slot-columns (= 7 * sum C)

# coefficient slot roles: 0:v0 1:v1 2:v2 3:v3 4:-v1 5:-v2 6:-v3
# output quarter <- sum of (slot, input quarter):
COMP_TERMS = {
    "a0r": [(0, "a0r"), (1, "a0i"), (5, "a1r"), (3, "a1i")],
    "a0i": [(4, "a0r"), (0, "a0i"), (6, "a1r"), (5, "a1i")],
    "a1r": [(2, "a0r"), (3, "a0i"), (0, "a1r"), (4, "a1i")],
    "a1i": [(6, "a0r"), (2, "a0i"), (1, "a1r"), (0, "a1i")],
}
QKEYS = {"a0r": (0, 0), "a0i": (1, 0), "a1r": (0, 1), "a1i": (1, 1)}


def _mk_ap(src_ap, dims):
    """Manual AP with explicit [stride, count] dims (partition dim first)."""
    return bass.AP(tensor=src_ap.tensor, offset=src_ap.offset,
                   ap=[list(d) for d in dims])


def _quarter_aps(st_ap, g, T):
    """Quarter-slice APs of state [128, 128, T] for CRot gate g.

    Returns dict name -> AP with dims [p, (A?), (Bm?), C*T] (c merged with
    the innermost tile dim; requires the state tile's last dim == T exactly).
    """
    cpos, tpos, hi, lo, A, Bm, C = _GEO[g]
    cbit_is_x = (cpos == hi)
    pdim = list(st_ap.ap[0])
    base_off = st_ap.offset
    # element strides within one partition (state tile is [128, 128, T],
    # contiguous): amp stride = T, tile stride = 1
    s_r = 64 * T
    s_a = 2 * Bm * 2 * C * T
    s_x = Bm * 2 * C * T
    s_b = 2 * C * T
    s_y = C * T
    out = {}
    for name, (r, tval) in QKEYS.items():
        xbit, ybit = (1, tval) if cbit_is_x else (tval, 1)
        off = base_off + r * s_r + xbit * s_x + ybit * s_y
        dims = [pdim]
        if A > 1:
            dims.append([s_a, A])
        if Bm > 1:
            dims.append([s_b, Bm])
        dims.append([1, C * T])
        out[name] = _mk_ap(st_ap, dims)
        out[name] = bass.AP(tensor=st_ap.tensor, offset=off,
                            ap=out[name].ap)
    return out


def _coef_aps(coef_ap, g, T):
    """Slot APs (broadcast to quarter shape) from replicated coef table
    [128, NREP, T].  Slot k of gate g occupies columns OFF[g]+k*C ..
    +C, real memory, so the merged (C*T) innermost dim is stride-1."""
    cpos, tpos, hi, lo, A, Bm, C = _GEO[g]
    pdim = list(coef_ap.ap[0])
    out = []
    for k in range(7):
        off = coef_ap.offset + (_OFF[g] + k * C) * T
        dims = [pdim]
        if A > 1:
            dims.append([0, A])
        if Bm > 1:
            dims.append([0, Bm])
        dims.append([1, C * T])
        out.append(bass.AP(tensor=coef_ap.tensor, offset=off, ap=dims))
    return out


def _emit_crot(nc, pool, cur, nxt, coef, g, T, tag, offload=False):
    """One CRot gate: read cur, write nxt (ping-pong), fp16 tile-minor.

    offload=True runs one of the four outputs on gpsimd (useful while
    gpsimd would otherwise idle, e.g. during the A chain)."""
    cpos, tpos, hi, lo, A, Bm, C = _GEO[g]
    qc = _quarter_aps(cur[:], g, T)
    qn = _quarter_aps(nxt[:], g, T)
    co = _coef_aps(coef[:], g, T)
    fsz = A * Bm * C * T                       # quarter free size
    for name, terms in COMP_TERMS.items():
        eng = nc.gpsimd if (offload and name == "a1i") else nc.vector
        tsuf = "g" if (offload and name == "a1i") else ""
        m1t = pool.tile([P, fsz], F16, tag=f"{tag}m1{tsuf}", name="m1t")
        m2t = pool.tile([P, fsz], F16, tag=f"{tag}m2{tsuf}", name="m2t")
        m3t = pool.tile([P, fsz], F16, tag=f"{tag}m3{tsuf}", name="m3t")
        shape_dims = qc[name].ap[1:]

        def shaped(tile_t):
            dims = [list(tile_t[:].ap[0])]
            stride = 1
            rev = []
            for d in reversed(shape_dims):
                rev.append([stride, d[1]])
                stride *= d[1]
            dims += rev[::-1]
            return bass.AP(tensor=tile_t[:].tensor, offset=tile_t[:].offset,
                           ap=dims)
        m1, m2, m3 = shaped(m1t), shaped(m2t), shaped(m3t)
        (s0, i0), (s1, i1), (s2, i2), (s3, i3) = terms
        eng.tensor_tensor(out=m1, in0=qc[i0], in1=co[s0], op=ALU.mult)
        eng.tensor_tensor(out=m2, in0=qc[i1], in1=co[s1], op=ALU.mult)
        eng.tensor_tensor(out=m1, in0=m1, in1=m2, op=ALU.add)
        eng.tensor_tensor(out=m2, in0=qc[i2], in1=co[s2], op=ALU.mult)
        eng.tensor_tensor(out=m3, in0=qc[i3], in1=co[s3], op=ALU.mult)
        eng.tensor_tensor(out=m2, in0=m2, in1=m3, op=ALU.add)
        eng.tensor_tensor(out=qn[name], in0=m1, in1=m2, op=ALU.add)
    # inactive (control=0) half: single merged copy cur -> nxt on ACT
    u = 64 >> cpos                              # dims above cpos incl r
    m = 1 << cpos
    s_c = m * T
    for st_ap, dst in ((cur[:], 0), (nxt[:], 1)):
        dims = [list(st_ap.ap[0])]
        if u > 1:
            dims.append([2 * s_c, u])
        dims.append([1, m * T])
        ap = bass.AP(tensor=st_ap.tensor, offset=st_ap.offset, ap=dims)
        if dst == 0:
            src_ap = ap
        else:
            dst_ap = ap
    nc.scalar.activation(out=dst_ap, in_=src_ap, func=ACTFN.Copy)


def _emit_doubling(nc, pool, sbufs, fac, T, tag):
    """Product state from factors, tile-minor.

    sbufs: (sA, sB) [128, 128, T] fp16.  fac: [128, 6, 6, T] fp16 with
    per-step slots [u0r, u0i, -u0i, u1r, u1i, -u1i]; step k expands wire
    q=5-k.  Returns the buffer holding the result (sA).
    """
    sA, sB = sbufs
    cur = sA
    for (dst_col, src_slot) in ((0, 0), (1, 3)):       # re: u0r, u1r
        nc.vector.tensor_copy(out=cur[:, dst_col, :],
                              in_=fac[:, 0, src_slot, :])
    for (dst_col, src_slot) in ((64, 1), (65, 4)):     # im: u0i, u1i
        nc.vector.tensor_copy(out=cur[:, dst_col, :],
                              in_=fac[:, 0, src_slot, :])
    for k in range(1, 6):
        w = 1 << k
        nxt = sB if cur is sA else sA
        fv = fac[:]

        def fpair(slot):
            # slots (slot, slot+3) for m=0/1: [p, m:2, w(bcast), T]
            off = fv.offset + (k * 6 + slot) * T
            return bass.AP(tensor=fv.tensor, offset=off,
                           ap=[list(fv.ap[0]), [3 * T, 2], [0, w], [1, T]])

        def mview(ap_base, col0):
            # [p, m:2, w, T] over state cols [col0, col0+2w)
            off = ap_base.offset + col0 * T
            return bass.AP(tensor=ap_base.tensor, offset=off,
                           ap=[list(ap_base.ap[0]), [w * T, 2], [T, w], [1, T]])

        def cbc(ap_base, col0):
            # [p, m-bcast:2, w, T] over cur cols [col0, col0+w)
            off = ap_base.offset + col0 * T
            return bass.AP(tensor=ap_base.tensor, offset=off,
                           ap=[list(ap_base.ap[0]), [0, 2], [T, w], [1, T]])
        crb, cib = cbc(cur[:], 0), cbc(cur[:], 64)
        dr, di = mview(nxt[:], 0), mview(nxt[:], 64)
        t1 = pool.tile([P, 2 * w * T], F16, tag=tag + "a")
        t2 = pool.tile([P, 2 * w * T], F16, tag=tag + "b")
        t1v = t1[:].rearrange("p (m w t) -> p m w t", m=2, w=w)
        t2v = t2[:].rearrange("p (m w t) -> p m w t", m=2, w=w)
        nc.vector.tensor_tensor(out=t1v, in0=crb, in1=fpair(0), op=ALU.mult)
        nc.vector.tensor_tensor(out=t2v, in0=cib, in1=fpair(2), op=ALU.mult)
        nc.vector.tensor_tensor(out=dr, in0=t1v, in1=t2v, op=ALU.add)
        nc.vector.tensor_tensor(out=t1v, in0=crb, in1=fpair(1), op=ALU.mult)
        nc.vector.tensor_tensor(out=t2v, in0=cib, in1=fpair(0), op=ALU.mult)
        nc.vector.tensor_tensor(out=di, in0=t1v, in1=t2v, op=ALU.add)
        cur = nxt
    if cur is not sA:
        nc.vector.tensor_copy(out=sA[:], in_=cur[:])
    return sA


def _emit_coef_replicate(nc, base, coef, T):
    """base [128, 18, 7, T] -> replicated coef [128, NREP, T]."""
    cv = coef[:]
    for g in range(18):
        C = _GEO[g][6]
        src = base[:, g, :, :].unsqueeze(2).to_broadcast([P, 7, C, T])
        dst = bass.AP(tensor=cv.tensor, offset=cv.offset + _OFF[g] * T,
                      ap=[list(cv.ap[0]), [C * T, 7], [T, C], [1, T]])
        nc.vector.tensor_copy(out=dst, in_=src)


_STAGES = ["prep", "dblA", "chainA", "storeT", "chainW", "storeW", "gather", "gatherw", "full"]


def build_program(stop_after=None, no_collective=False):
    lim = _STAGES.index(stop_after) if stop_after else len(_STAGES)

    def on(stage):
        return _STAGES.index(stage) < lim or stage == stop_after
    nc = bacc.Bacc("TRN2", target_bir_lowering=False, debug=False,
                   num_swdge_queues=4)

    ent = nc.dram_tensor("ent_par", [ETILES, P, 72], F32, kind="ExternalInput")
    wcoef_d = nc.dram_tensor("wcoefb", [P, 18, 7, WT], F16, kind="ExternalInput")
    wfac_d = nc.dram_tensor("wfac", [P, 6, 6, WT], F16, kind="ExternalInput")
    sidx_d = nc.dram_tensor("sidx", [P, NIDX_T // 16], I16, kind="ExternalInput")
    oidx_d = nc.dram_tensor("oidx", [P, NIDX_T // 16], I16, kind="ExternalInput")
    scores_d = nc.dram_tensor("scores", [1, NST * STW], F32, kind="ExternalOutput")
    dbg_d = nc.dram_tensor("dbg", [P, 8192], F16, kind="ExternalOutput") if stop_after else None

    with tile.TileContext(nc) as tc:
        with (
            tc.tile_pool(name="const", bufs=1) as cp,
            tc.tile_pool(name="gtmp", bufs=2) as gp,
            tc.tile_pool(name="state", bufs=1) as sp,
            tc.tile_pool(name="cbuf", bufs=1) as cb,
            tc.tile_pool(name="prodb", bufs=3) as pb,
            tc.tile_pool(name="cpy", bufs=2, space="PSUM") as psY,
            tc.tile_pool(name="cpsc", bufs=2, space="PSUM") as psS,
            tc.tile_pool(name="dram", bufs=1, space="DRAM") as dp,
        ):
            # ---------------- DRAM scratch ----------------
            T_loc = dp.tile([EPC, P], F16)
            T_full = dp.tile([EPAD, P], F16, addr_space="Shared")
            W_loc = dp.tile([P * RSLOT, P], F16)   # row = j*RSLOT + slot

            # ---------------- inputs ----------------
            ang = cp.tile([P, ETILES, 72], F32)
            nc.sync.dma_start(out=ang[:], in_=ent[:].rearrange("t p k -> p t k"))
            wcoefb = cp.tile([P, 18, 7, WT], F16)
            nc.sync.dma_start(out=wcoefb[:], in_=wcoef_d[:])
            facW = cp.tile([P, 6, 6, WT], F16)
            nc.sync.dma_start(out=facW[:], in_=wfac_d[:])
            sidx = cp.tile([P, NIDX_T // 16], I16)
            nc.sync.dma_start(out=sidx[:], in_=sidx_d[:])
            oidx = cp.tile([P, NIDX_T // 16], I16)
            nc.sync.dma_start(out=oidx[:], in_=oidx_d[:])

            ones = cp.tile([P, 1], F16)
            nc.vector.memset(ones[:], 1.0)

            # const APs for activation scale/bias floats
            cdb = cp.tile([P, 3], F32)
            nc.vector.memset(cdb[:, 0:1], 0.0)
            nc.vector.memset(cdb[:, 1:2], 0.5)
            nc.vector.memset(cdb[:, 2:3], PI / 2)
            nc.const_aps.aps[(F32, 0.0)] = cdb[:, 0:1]
            nc.const_aps.aps[(F32, 0.5)] = cdb[:, 1:2]
            nc.const_aps.aps[(F32, PI / 2)] = cdb[:, 2:3]

            # ---------------- A: entity angle prep ----------------
            TA = ETILES
            angT = cp.tile([P, 72, TA], F32)     # tile-minor angles
            nc.vector.tensor_copy(
                out=angT[:], in_=ang[:].rearrange("p t k -> p k t"))
            gv = angT[:].rearrange("p (g a) t -> p g a t", g=24, a=3)
            phi, tha, omg = gv[:, :, 0, :], gv[:, :, 1, :], gv[:, :, 2, :]
            s1 = cp.tile([P, 24, TA], F32)
            s2 = cp.tile([P, 24, TA], F32)
            nc.vector.tensor_tensor(out=s1[:], in0=phi, in1=omg, op=ALU.add)
            nc.vector.tensor_tensor(out=s2[:], in0=phi, in1=omg, op=ALU.subtract)

            half = cp.tile([P, 6, 24, TA], F32)
            trig = cp.tile([P, 6, 24, TA], F32)  # ch sh ca sa cb sb
            hv, tv = half[:], trig[:]
            for i, srcv in ((0, tha), (2, s1[:]), (4, s2[:])):
                nc.vector.tensor_scalar(
                    out=hv[:, i], in0=srcv, scalar1=0.5, scalar2=PI / 2,
                    op0=ALU.mult, op1=ALU.add)
                nc.vector.tensor_scalar_mul(hv[:, i + 1], srcv, 0.5)
            for i in range(6):
                nc.scalar.activation(out=tv[:, i], in_=hv[:, i], func=ACTFN.Sin)

            # products v0..v3 for all 24 gates, f32
            vprod = cp.tile([P, 4, 24, TA], F32)
            vv = vprod[:]
            nc.vector.tensor_tensor(out=vv[:, 0], in0=tv[:, 0], in1=tv[:, 2], op=ALU.mult)
            nc.vector.tensor_tensor(out=vv[:, 1], in0=tv[:, 0], in1=tv[:, 3], op=ALU.mult)
            nc.vector.tensor_tensor(out=vv[:, 2], in0=tv[:, 1], in1=tv[:, 4], op=ALU.mult)
            nc.vector.tensor_tensor(out=vv[:, 3], in0=tv[:, 1], in1=tv[:, 5], op=ALU.mult)

            # base CRot coef table [p, 18, 7, T] fp16 (chain gates 6..23)
            abase = cp.tile([P, 18, 7, TA], F16)
            ab = abase[:]
            for k in range(4):
                nc.vector.tensor_copy(out=ab[:, :, k, :], in_=vv[:, k, 6:24, :])
            for dst, src in ((4, 1), (5, 2), (6, 3)):
                nc.vector.tensor_scalar_mul(ab[:, :, dst, :], vv[:, src, 6:24, :], -1.0)

            # layer-0 doubling factors facA [p, 6 steps, 6 slots, T]
            # step k expands wire q=5-k -> uses layer-0 gate qg=5-k
            # f0r=(v0-v2)r2  f0i=-(v1+v3)r2  f1r=(v0+v2)r2  f1i=(v1-v3)r2
            facA = cp.tile([P, 6, 6, TA], F16)
            t6 = cp.tile([P, 6, TA], F32)
            t6v = t6[:]
            fav = facA[:]

            def fac_dst(slot):
                # k dim reversed: k = 5 - g  (g = gate 0..5)
                base = fav.offset + 5 * (6 * TA) + slot * TA
                return bass.AP(tensor=fav.tensor, offset=base,
                               ap=[list(fav.ap[0]), [-6 * TA, 6], [1, TA]])
            g03 = vv[:, 0, 0:6, :], vv[:, 1, 0:6, :], vv[:, 2, 0:6, :], vv[:, 3, 0:6, :]
            v0g, v1g, v2g, v3g = g03
            nc.vector.tensor_tensor(out=t6v, in0=v0g, in1=v2g, op=ALU.subtract)
            nc.vector.tensor_scalar_mul(fac_dst(0), t6v, R2)
            nc.vector.tensor_tensor(out=t6v, in0=v1g, in1=v3g, op=ALU.add)
            nc.vector.tensor_scalar_mul(fac_dst(1), t6v, -R2)
            nc.vector.tensor_scalar_mul(fac_dst(2), t6v, R2)
            nc.vector.tensor_tensor(out=t6v, in0=v0g, in1=v2g, op=ALU.add)
            nc.vector.tensor_scalar_mul(fac_dst(3), t6v, R2)
            nc.vector.tensor_tensor(out=t6v, in0=v1g, in1=v3g, op=ALU.subtract)
            nc.vector.tensor_scalar_mul(fac_dst(4), t6v, R2)
            nc.vector.tensor_scalar_mul(fac_dst(5), t6v, -R2)

            # replicated coef tables
            acoef = cp.tile([P, NREP, TA], F16)
            _emit_coef_replicate(nc, abase, acoef, TA)
            wcoef = cp.tile([P, NREP, WT], F16)
            _emit_coef_replicate(nc, wcoefb, wcoef, WT)

            if stop_after == "prep":
                nc.sync.dma_start(out=dbg_d[:, 0:NREP * TA],
                                  in_=acoef[:].rearrange("p a t -> p (a t)"))
                nc.sync.dma_start(out=dbg_d[:, NREP * TA:NREP * TA + 360],
                                  in_=facA[:].rearrange("p a b t -> p (a b t)"))
            # ---------------- A chain ----------------
            if on("dblA"):
                sA0 = sp.tile([P, P, TA], F16)
                sA1 = sp.tile([P, P, TA], F16)
                cur = _emit_doubling(nc, gp, (sA0, sA1), facA[:], TA, "adbl")
                nxt = sA1
            if stop_after == "dblA":
                nc.sync.dma_start(out=dbg_d[:, 0:P * TA],
                                  in_=cur[:].rearrange("p a t -> p (a t)"))
            if on("chainA"):
                for g in range(18):
                    _emit_crot(nc, gp, cur, nxt, acoef, g, TA, "acr",
                               offload=True)
                    cur, nxt = nxt, cur
            if stop_after == "chainA":
                nc.sync.dma_start(out=dbg_d[:, 0:P * TA],
                                  in_=cur[:].rearrange("p a t -> p (a t)"))
            if on("storeT"):
                # T store (transpose to row-major) + AllGather
                Tout = cp.tile([P, TA, P], F16)
                nc.scalar.activation(
                    out=Tout[:], in_=cur[:].rearrange("p a t -> p t a"),
                    func=ACTFN.Copy)
                nc.sync.dma_start(
                    out=T_loc[:].rearrange("(t p) k -> p t k", p=P), in_=Tout[:])
                if no_collective:
                    nc.sync.dma_start(out=T_full[0:EPC, :], in_=T_loc[:])
                else:
                    nc.gpsimd.collective_compute(
                        "AllGather", ALU.bypass,
                        ins=[T_loc[:]], outs=[T_full[:]],
                        replica_groups=[list(range(NCORES))],
                    )

            if stop_after == "storeT":
                nc.sync.dma_start(out=dbg_d[:, 0:TA * P],
                                  in_=Tout[:].rearrange("p a t -> p (a t)"))

            # ---------------- W chain ----------------
            if on("chainW"):
                sW0 = sp.tile([P, P, WT], F16)
                sW1 = sp.tile([P, P, WT], F16)
                curw = _emit_doubling(nc, gp, (sW0, sW1), facW[:], WT, "wdbl")
                nxtw = sW1
                for g in range(18):
                    _emit_crot(nc, gp, curw, nxtw, wcoef, g, WT, "wcr")
                    curw, nxtw = nxtw, curw
            if stop_after == "chainW":
                nc.sync.dma_start(out=dbg_d[:, 0:P * WT],
                                  in_=curw[:].rearrange("p a t -> p (a t)"))

            # expand to W^T rows.  state partition p=(h,j): column j of W for
            # slot 2t+h, values [yr(64)|yi(64)].  W^T row j = state row;
            # W^T row 64+j = [-yi | yr].
            if on("storeW"):
                Wt = cp.tile([P, WT, P], F16)
                nc.scalar.activation(
                    out=Wt[:], in_=curw[:].rearrange("p a t -> p t a"),
                    func=ACTFN.Copy)
                Bt = cp.tile([P, WT, P], F16)
                nc.vector.tensor_scalar_mul(Bt[:, :, 0:64], Wt[:, :, 64:128], -1.0)
                nc.vector.tensor_copy(out=Bt[:, :, 64:128], in_=Wt[:, :, 0:64])
                # W_loc row = j*RSLOT + (2t+h);  p = h*64+j
                wl = W_loc[:].rearrange("(j t2 h) i -> h j t2 i", j=P, t2=WT, h=2)
                for h in (0, 1):
                    nc.sync.dma_start(out=wl[h, 0:64], in_=Wt[64 * h:64 * h + 64])
                    nc.sync.dma_start(out=wl[h, 64:128], in_=Bt[64 * h:64 * h + 64])
                # static supertile<->slot schedule (st == slot): whole W table
                # back to SBUF, no per-supertile gather needed.
                W_all = cp.tile([P, RSLOT, P], F16)
                nc.sync.dma_start(
                    out=W_all[:],
                    in_=W_loc[:].rearrange("(j s) i -> j s i", s=RSLOT))


            # ---------------- phase C gathers (chunked; SWDGE ring caps
            # one gather at ~896 idxs) ----------------
            GCH = 896
            qn = [0]

            def gather_chunks(out3, idxs_t, src, nidx, transpose):
                s = 0
                while s < nidx:
                    n = min(GCH, nidx - s)
                    if transpose:
                        o = out3[:, :, s:s + n]
                    else:
                        o = out3[:, s // P:(s + n) // P, :]
                    nc.gpsimd.dma_gather(
                        out_ap=o, in_ap=src, idxs_ap=idxs_t[:, s // 16:(s + n) // 16],
                        num_idxs=n, num_idxs_reg=n, elem_size=P,
                        transpose=transpose, queue_num=0)
                    qn[0] = 0
                    s += n

            SUBS = [7, 7, 7, 5]                # supertiles per subtile
            if on("gather"):
              TsTs, ToTs = [], []
              for si, nst in enumerate(SUBS):
                  tsub = cb.tile([P, 1, nst * STW], F16, tag=f"tst{si}", name=f"TsT{si}")
                  osub = cb.tile([P, 1, nst * STW], F16, tag=f"tot{si}", name=f"ToT{si}")
                  TsTs.append(tsub)
                  ToTs.append(osub)
              base = 0
              for si, nst in enumerate(SUBS):
                  nid = nst * STW
                  s = 0
                  while s < nid:
                      n = min(GCH, nid - s)
                      for (buf, idxs_t) in ((TsTs[si], sidx), (ToTs[si], oidx)):
                          nc.gpsimd.dma_gather(
                              out_ap=buf[:, :, s:s + n], in_ap=T_full[:],
                              idxs_ap=idxs_t[:, (base + s) // 16:(base + s + n) // 16],
                              num_idxs=n, num_idxs_reg=n, elem_size=P,
                              transpose=True, queue_num=0)
                      s += n
                  base += nid
            if stop_after == "gather":
                nc.sync.dma_start(out=dbg_d[:, 0:2048],
                                  in_=TsTs[0][:, 0, 0:2048])
            if stop_after == "gatherw":
                nc.sync.dma_start(out=dbg_d[:, 0:3328],
                                  in_=W_all[:].rearrange("p a b -> p (a b)"))

            if on("full"):
              scores = cp.tile([1, NST * STW], F32)
              st = 0
              for si, nst in enumerate(SUBS):
                for li in range(nst):
                 pY = psY.tile([P, STW], F32, tag="py")
                 nc.tensor.matmul(out=pY[:], lhsT=W_all[:, st, :],
                                  rhs=TsTs[si][:, 0, li * STW:(li + 1) * STW],
                                  start=True, stop=True)
                 prod = pb.tile([P, STW], F16, tag="prod")
                 nc.vector.tensor_tensor(
                     out=prod[:], in0=pY[:],
                     in1=ToTs[si][:, 0, li * STW:(li + 1) * STW], op=ALU.mult)
                 pS = psS.tile([1, STW], F32, tag="ps")
                 nc.tensor.matmul(out=pS[:], lhsT=ones[:], rhs=prod[:],
                                  start=True, stop=True)
                 nc.scalar.activation(
                     out=scores[0:1, st * STW:(st + 1) * STW], in_=pS[:],
                     func=ACTFN.Copy)
                 st += 1
              nc.sync.dma_start(out=scores_d[:], in_=scores[:])

    nc.finalize()
    return nc


# --------------------------------------------------------------------------
# host side
# --------------------------------------------------------------------------

def _rot_elems(params):
    """params [..., 3] (phi, theta, omega) -> v0..v3 f32.

    m00=(v0,-v1) m01=(-v2,-v3) m10=(v2,-v3) m11=(v0,v1)
    """
    phi, tha, omg = params[..., 0], params[..., 1], params[..., 2]
    ch, sh = np.cos(tha / 2), np.sin(tha / 2)
    a, b = (phi + omg) / 2, (phi - omg) / 2
    return (
        (ch * np.cos(a)).astype(np.float32),
        (ch * np.sin(a)).astype(np.float32),
        (sh * np.cos(b)).astype(np.float32),
        (sh * np.sin(b)).astype(np.float32),
    )


def _pack_idxs(flat):
    """int array [n] (n % 16 == 0) -> [128, n/16] int16 (replicated x8)."""
    n = len(flat)
    blk = np.asarray(flat, np.int16).reshape(n // 16, 16).T
    return np.tile(blk, (8, 1))


def _host_prep(entity_params, relation_params, s_idx, p_idx, o_idx):
    ent = np.asarray(entity_params, dtype=np.float32)
    rel = np.asarray(relation_params, dtype=np.float32)
    s_idx = np.asarray(s_idx)
    p_idx = np.asarray(p_idx)
    o_idx = np.asarray(o_idx)

    # ---- entity shards ----
    ent_flat = ent.reshape(E, 72)
    ent_pad = np.zeros((EPAD, 72), np.float32)
    ent_pad[:E] = ent_flat
    ent_shards = [
        ent_pad[c * EPC:(c + 1) * EPC].reshape(ETILES, P, 72)
        for c in range(NCORES)
    ]

    # ---- supertiles (relation, <=512 elements) ----
    order = np.argsort(p_idx, kind="stable")
    bounds = np.searchsorted(p_idx[order], np.arange(R + 1))
    tiles = []
    for r in range(R):
        grp = order[bounds[r]:bounds[r + 1]]
        for i in range(0, len(grp), STW):
            tiles.append((r, grp[i:i + STW]))
    ntiles = len(tiles)
    assert ntiles <= NCORES * NST, f"too many supertiles {ntiles}"

    # greedy balance: relations (grouped) to cores, bounded slots+tiles
    by_rel = {}
    for t in tiles:
        by_rel.setdefault(t[0], []).append(t)
    core_tiles = [[] for _ in range(NCORES)]
    for r, ts in sorted(by_rel.items(), key=lambda kv: -len(kv[1])):
        remaining = list(ts)
        while remaining:
            c = min(range(NCORES), key=lambda cc: len(core_tiles[cc]))
            cap = NST - len(core_tiles[c])
            assert cap > 0, "balance failure"
            take = remaining[:cap]
            core_tiles[c].extend(take)
            remaining = remaining[len(take):]

    gate_rel = rel.reshape(R, 24, 3)
    v0, v1, v2, v3 = _rot_elems(gate_rel)   # each [R, 24]

    in_maps = []
    outpos = np.full((NCORES, NST, STW), -1, np.int64)
    for c in range(NCORES):
        ct = core_tiles[c]
        assert len(ct) <= NST
        # static schedule: slot st computes supertile st's relation
        slot_rel = [ct[t][0] if t < len(ct) else 0 for t in range(RSLOT)]

        sflat = np.zeros(NIDX_T, np.int64)
        oflat = np.zeros(NIDX_T, np.int64)
        for t in range(NST):
            if t < len(ct):
                r, elems = ct[t]
                n = len(elems)
                sflat[t * STW:t * STW + n] = s_idx[elems]
                oflat[t * STW:t * STW + n] = o_idx[elems]
                outpos[c, t, :n] = elems

        # W-chain coef [p, 18, 7, WT] fp16 + doubling factors [p,6,6,WT]
        wcoefb = np.zeros((P, 18, 7, WT), np.float16)
        wfac = np.zeros((P, 6, 6, WT), np.float16)
        for sl, r in enumerate(slot_rel):
            wt, hh = divmod(sl, 2)
            rows = slice(64 * hh, 64 * hh + 64)
            for g in range(18):
                gg = 6 + g
                vals = (v0[r, gg], v1[r, gg], v2[r, gg], v3[r, gg],
                        -v1[r, gg], -v2[r, gg], -v3[r, gg])
                for k, vvv in enumerate(vals):
                    wcoefb[rows, g, k, wt] = vvv
            j = np.arange(64)
            for k in range(6):
                qg = 5 - k
                bit = (j >> k) & 1
                m00 = (v0[r, qg], -v1[r, qg])
                m01 = (-v2[r, qg], -v3[r, qg])
                m10 = (v2[r, qg], -v3[r, qg])
                m11 = (v0[r, qg], v1[r, qg])
                u0r = np.where(bit == 0, m00[0], m01[0])
                u0i = np.where(bit == 0, m00[1], m01[1])
                u1r = np.where(bit == 0, m10[0], m11[0])
                u1i = np.where(bit == 0, m10[1], m11[1])
                wfac[rows, k, 0, wt] = u0r
                wfac[rows, k, 1, wt] = u0i
                wfac[rows, k, 2, wt] = -u0i
                wfac[rows, k, 3, wt] = u1r
                wfac[rows, k, 4, wt] = u1i
                wfac[rows, k, 5, wt] = -u1i

        in_maps.append({
            "ent_par": ent_shards[c],
            "wcoefb": wcoefb,
            "wfac": wfac,
            "sidx": _pack_idxs(sflat),
            "oidx": _pack_idxs(oflat),
        })
    return in_maps, outpos


_PROGRAM = None


def kernel(entity_params, relation_params, s_idx, p_idx, o_idx):
    global _PROGRAM
    in_maps, outpos = _host_prep(entity_params, relation_params,
                                 s_idx, p_idx, o_idx)
    if _PROGRAM is None:
        _PROGRAM = build_program()
    nc = _PROGRAM
    res = run_bass_kernel_spmd(nc, in_maps, list(range(NCORES)))
    out = np.zeros(B, np.float32)
    for c in range(NCORES):
        sc = res.results[c]["scores"].reshape(NST, STW)
        pos = outpos[c]
        mask = pos >= 0
        out[pos[mask]] = sc[mask]
    return out


if __name__ == "__main__":
    build_program()
    print("build OK")


# revision 22
# speedup vs baseline: 1.0399x; 1.0399x over previous
"""Trainium2 Bass kernel for the quantum-circuit KG-embedding scoring model.

score(s,p,o) = Re(<B_o h | W_p | B_s h>), B_e / W_p = 24-gate circuit blocks,
h = |+>^6.  State dim 64 complex = 128 reals [re(64) | im(64)].

Device algorithm (8 cores, SPMD), fp16 throughout the heavy paths:

  Chain phase (DVE, fp16 tile-minor layout [128 ent, 128 amp-real, T]):
    A-chain: evolve 1280 entities/core (10 tiles innermost) through the
    entity block (product-state doubling + 18 CRots).  Every DVE op has a
    stride-1 fp16 innermost dim -> 2x perf mode; coefficient tables are
    pre-replicated over the gate's low amp bits so no op exceeds 3 free
    dims (TENSOR3D) and no gate needs group splitting.
    Store T rows fp16 -> AllGather -> T_full [10240,128] fp16.
    W-chain: same machinery on 13 tiles = 26 relation slots x 64 basis
    columns, overlapping the AllGather.  Expand to W^T rows in DRAM.

  Phase C (supertiles of 512 elements, one relation each):
    dma_gather(transpose=True) pulls Ts^T / To^T [128 state, 13312 elems]
    straight from T_full (0.34ns/descriptor on gpsimd vs ~12ns for
    per-row indirect DMA; transfers spread over 16 DMA engines).
    dma_gather pulls per-supertile W^T [128,128] tiles.
    Per supertile: PE fp16 matmul Y = W @ Ts^T (psum f32), DVE
    prod = Y * To^T (fp16), PE ones-matmul column-sum -> scores.

Host does only: trig for the 200-relation tables, index sort/packing,
output unpermute (same division of labour as the reference baseline).
"""

import sys
import numpy as np

for _p in ("/opt/trn_rl_repo",):
    if _p not in sys.path:
        sys.path.insert(0, _p)

import concourse.bass as bass
import concourse.bacc as bacc
import concourse.mybir as mybir
from concourse import tile
from concourse.bass_utils import run_bass_kernel_spmd

F32 = mybir.dt.float32
F16 = mybir.dt.float16
I16 = mybir.dt.int16
ALU = mybir.AluOpType
ACTFN = mybir.ActivationFunctionType

P = 128
Q = 6
NA = 64                      # 2^Q amplitudes
NCORES = 8
E, R, B = 10000, 200, 65536
ETILES = 10                  # entity tiles per core
EPC = ETILES * P             # 1280 entities per core
EPAD = EPC * NCORES          # 10240 padded entity rows
WT = 13                      # W-chain tiles per core (2 rel slots each)
RSLOT = 2 * WT               # 26 relation slots per core
NST = 26                     # phase-C supertiles per core
STW = 512                    # supertile width (elements)
NIDX_T = NST * STW           # 13312 gathered T rows per table per core
NIDX_W = NST * P             # 3328 gathered W^T rows per core
R2 = float(2.0 ** -0.5)
PI = float(np.pi)

# CRot gate list: (control, target) wire pairs, in circuit order
CROTS = [(q, (q + off) % Q) for off in (1, 2, 3) for q in range(Q)]

# per-gate geometry: amp bit positions cpos=5-c (control), tpos=5-t
_GEO = []
for (c, t) in CROTS:
    cpos, tpos = 5 - c, 5 - t
    hi, lo = max(cpos, tpos), min(cpos, tpos)
    A = 1 << (5 - hi)
    Bm = 1 << (hi - lo - 1)
    C = 1 << lo
    _GEO.append((cpos, tpos, hi, lo, A, Bm, C))

# replicated coefficient table layout: per gate, 7 slots each replicated C
# times: block [7, C, T] at offset OFF[g] (in slot-columns of width T)
_OFF = []
_o = 0
for (_, _, _, _, _, _, C) in _GEO:
    _OFF.append(_o)
    _o += 7 * C
NREP = _o                    # total replicated slot-columns (= 7 * sum C)

# coefficient slot roles: 0:v0 1:v1 2:v2 3:v3 4:-v1 5:-v2 6:-v3
# output quarter <- sum of (slot, input quarter):
COMP_TERMS = {
    "a0r": [(0, "a0r"), (1, "a0i"), (5, "a1r"), (3, "a1i")],
    "a0i": [(4, "a0r"), (0, "a0i"), (6, "a1r"), (5, "a1i")],
    "a1r": [(2, "a0r"), (3, "a0i"), (0, "a1r"), (4, "a1i")],
    "a1i": [(6, "a0r"), (2, "a0i"), (1, "a1r"), (0, "a1i")],
}
QKEYS = {"a0r": (0, 0), "a0i": (1, 0), "a1r": (0, 1), "a1i": (1, 1)}


def _mk_ap(src_ap, dims):
    """Manual AP with explicit [stride, count] dims (partition dim first)."""
    return bass.AP(tensor=src_ap.tensor, offset=src_ap.offset,
                   ap=[list(d) for d in dims])


def _quarter_aps(st_ap, g, T):
    """Quarter-slice APs of state [128, 128, T] for CRot gate g.

    Returns dict name -> AP with dims [p, (A?), (Bm?), C*T] (c merged with
    the innermost tile dim; requires the state tile's last dim == T exactly).
    """
    cpos, tpos, hi, lo, A, Bm, C = _GEO[g]
    cbit_is_x = (cpos == hi)
    pdim = list(st_ap.ap[0])
    base_off = st_ap.offset
    # element strides within one partition (state tile is [128, 128, T],
    # contiguous): amp stride = T, tile stride = 1
    s_r = 64 * T
    s_a = 2 * Bm * 2 * C * T
    s_x = Bm * 2 * C * T
    s_b = 2 * C * T
    s_y = C * T
    out = {}
    for name, (r, tval) in QKEYS.items():
        xbit, ybit = (1, tval) if cbit_is_x else (tval, 1)
        off = base_off + r * s_r + xbit * s_x + ybit * s_y
        dims = [pdim]
        if A > 1:
            dims.append([s_a, A])
        if Bm > 1:
            dims.append([s_b, Bm])
        dims.append([1, C * T])
        out[name] = _mk_ap(st_ap, dims)
        out[name] = bass.AP(tensor=st_ap.tensor, offset=off,
                            ap=out[name].ap)
    return out


def _coef_aps(coef_ap, g, T):
    """Slot APs (broadcast to quarter shape) from replicated coef table
    [128, NREP, T].  Slot k of gate g occupies columns OFF[g]+k*C ..
    +C, real memory, so the merged (C*T) innermost dim is stride-1."""
    cpos, tpos, hi, lo, A, Bm, C = _GEO[g]
    pdim = list(coef_ap.ap[0])
    out = []
    for k in range(7):
        off = coef_ap.offset + (_OFF[g] + k * C) * T
        dims = [pdim]
        if A > 1:
            dims.append([0, A])
        if Bm > 1:
            dims.append([0, Bm])
        dims.append([1, C * T])
        out.append(bass.AP(tensor=coef_ap.tensor, offset=off, ap=dims))
    return out


def _emit_crot(nc, pool, cur, nxt, coef, g, T, tag, offload=False):
    """One CRot gate: read cur, write nxt (ping-pong), fp16 tile-minor.

    offload=True runs one of the four outputs on gpsimd (useful while
    gpsimd would otherwise idle, e.g. during the A chain)."""
    cpos, tpos, hi, lo, A, Bm, C = _GEO[g]
    qc = _quarter_aps(cur[:], g, T)
    qn = _quarter_aps(nxt[:], g, T)
    co = _coef_aps(coef[:], g, T)
    fsz = A * Bm * C * T                       # quarter free size
    for name, terms in COMP_TERMS.items():
        eng = nc.gpsimd if (offload and name == "a1i") else nc.vector
        tsuf = "g" if (offload and name == "a1i") else ""
        m1t = pool.tile([P, fsz], F16, tag=f"{tag}m1{tsuf}", name="m1t")
        m2t = pool.tile([P, fsz], F16, tag=f"{tag}m2{tsuf}", name="m2t")
        m3t = pool.tile([P, fsz], F16, tag=f"{tag}m3{tsuf}", name="m3t")
        shape_dims = qc[name].ap[1:]

        def shaped(tile_t):
            dims = [list(tile_t[:].ap[0])]
            stride = 1
            rev = []
            for d in reversed(shape_dims):
                rev.append([stride, d[1]])
                stride *= d[1]
            dims += rev[::-1]
            return bass.AP(tensor=tile_t[:].tensor, offset=tile_t[:].offset,
                           ap=dims)
        m1, m2, m3 = shaped(m1t), shaped(m2t), shaped(m3t)
        (s0, i0), (s1, i1), (s2, i2), (s3, i3) = terms
        eng.tensor_tensor(out=m1, in0=qc[i0], in1=co[s0], op=ALU.mult)
        eng.tensor_tensor(out=m2, in0=qc[i1], in1=co[s1], op=ALU.mult)
        eng.tensor_tensor(out=m1, in0=m1, in1=m2, op=ALU.add)
        eng.tensor_tensor(out=m2, in0=qc[i2], in1=co[s2], op=ALU.mult)
        eng.tensor_tensor(out=m3, in0=qc[i3], in1=co[s3], op=ALU.mult)
        eng.tensor_tensor(out=m2, in0=m2, in1=m3, op=ALU.add)
        eng.tensor_tensor(out=qn[name], in0=m1, in1=m2, op=ALU.add)
    # inactive (control=0) half: single merged copy cur -> nxt on ACT
    u = 64 >> cpos                              # dims above cpos incl r
    m = 1 << cpos
    s_c = m * T
    for st_ap, dst in ((cur[:], 0), (nxt[:], 1)):
        dims = [list(st_ap.ap[0])]
        if u > 1:
            dims.append([2 * s_c, u])
        dims.append([1, m * T])
        ap = bass.AP(tensor=st_ap.tensor, offset=st_ap.offset, ap=dims)
        if dst == 0:
            src_ap = ap
        else:
            dst_ap = ap
    nc.scalar.activation(out=dst_ap, in_=src_ap, func=ACTFN.Copy)


def _emit_doubling(nc, pool, sbufs, fac, T, tag):
    """Product state from factors, tile-minor.

    sbufs: (sA, sB) [128, 128, T] fp16.  fac: [128, 6, 6, T] fp16 with
    per-step slots [u0r, u0i, -u0i, u1r, u1i, -u1i]; step k expands wire
    q=5-k.  Returns the buffer holding the result (sA).
    """
    sA, sB = sbufs
    cur = sA
    for (dst_col, src_slot) in ((0, 0), (1, 3)):       # re: u0r, u1r
        nc.vector.tensor_copy(out=cur[:, dst_col, :],
                              in_=fac[:, 0, src_slot, :])
    for (dst_col, src_slot) in ((64, 1), (65, 4)):     # im: u0i, u1i
        nc.vector.tensor_copy(out=cur[:, dst_col, :],
                              in_=fac[:, 0, src_slot, :])
    for k in range(1, 6):
        w = 1 << k
        nxt = sB if cur is sA else sA
        fv = fac[:]

        def fpair(slot):
            # slots (slot, slot+3) for m=0/1: [p, m:2, w(bcast), T]
            off = fv.offset + (k * 6 + slot) * T
            return bass.AP(tensor=fv.tensor, offset=off,
                           ap=[list(fv.ap[0]), [3 * T, 2], [0, w], [1, T]])

        def mview(ap_base, col0):
            # [p, m:2, w, T] over state cols [col0, col0+2w)
            off = ap_base.offset + col0 * T
            return bass.AP(tensor=ap_base.tensor, offset=off,
                           ap=[list(ap_base.ap[0]), [w * T, 2], [T, w], [1, T]])

        def cbc(ap_base, col0):
            # [p, m-bcast:2, w, T] over cur cols [col0, col0+w)
            off = ap_base.offset + col0 * T
            return bass.AP(tensor=ap_base.tensor, offset=off,
                           ap=[list(ap_base.ap[0]), [0, 2], [T, w], [1, T]])
        crb, cib = cbc(cur[:], 0), cbc(cur[:], 64)
        dr, di = mview(nxt[:], 0), mview(nxt[:], 64)
        t1 = pool.tile([P, 2 * w * T], F16, tag=tag + "a")
        t2 = pool.tile([P, 2 * w * T], F16, tag=tag + "b")
        t1v = t1[:].rearrange("p (m w t) -> p m w t", m=2, w=w)
        t2v = t2[:].rearrange("p (m w t) -> p m w t", m=2, w=w)
        nc.vector.tensor_tensor(out=t1v, in0=crb, in1=fpair(0), op=ALU.mult)
        nc.vector.tensor_tensor(out=t2v, in0=cib, in1=fpair(2), op=ALU.mult)
        nc.vector.tensor_tensor(out=dr, in0=t1v, in1=t2v, op=ALU.add)
        nc.vector.tensor_tensor(out=t1v, in0=crb, in1=fpair(1), op=ALU.mult)
        nc.vector.tensor_tensor(out=t2v, in0=cib, in1=fpair(0), op=ALU.mult)
        nc.vector.tensor_tensor(out=di, in0=t1v, in1=t2v, op=ALU.add)
        cur = nxt
    if cur is not sA:
        nc.vector.tensor_copy(out=sA[:], in_=cur[:])
    return sA


def _emit_coef_replicate(nc, base, coef, T):
    """base [128, 18, 7, T] -> replicated coef [128, NREP, T]."""
    cv = coef[:]
    for g in range(18):
        C = _GEO[g][6]
        src = base[:, g, :, :].unsqueeze(2).to_broadcast([P, 7, C, T])
        dst = bass.AP(tensor=cv.tensor, offset=cv.offset + _OFF[g] * T,
                      ap=[list(cv.ap[0]), [C * T, 7], [T, C], [1, T]])
        nc.vector.tensor_copy(out=dst, in_=src)


_STAGES = ["prep", "dblA", "chainA", "storeT", "chainW", "storeW", "gather", "gatherw", "full"]


def build_program(stop_after=None, no_collective=False):
    lim = _STAGES.index(stop_after) if stop_after else len(_STAGES)

    def on(stage):
        return _STAGES.index(stage) < lim or stage == stop_after
    nc = bacc.Bacc("TRN2", target_bir_lowering=False, debug=False,
                   num_swdge_queues=4)

    ent = nc.dram_tensor("ent_par", [ETILES, P, 72], F32, kind="ExternalInput")
    wcoef_d = nc.dram_tensor("wcoefb", [P, 18, 7, WT], F16, kind="ExternalInput")
    wfac_d = nc.dram_tensor("wfac", [P, 6, 6, WT], F16, kind="ExternalInput")
    sidx_d = nc.dram_tensor("sidx", [P, NIDX_T // 16], I16, kind="ExternalInput")
    oidx_d = nc.dram_tensor("oidx", [P, NIDX_T // 16], I16, kind="ExternalInput")
    scores_d = nc.dram_tensor("scores", [1, NST * STW], F32, kind="ExternalOutput")
    dbg_d = nc.dram_tensor("dbg", [P, 8192], F16, kind="ExternalOutput") if stop_after else None

    with tile.TileContext(nc) as tc:
        with (
            tc.tile_pool(name="const", bufs=1) as cp,
            tc.tile_pool(name="gtmp", bufs=2) as gp,
            tc.tile_pool(name="state", bufs=1) as sp,
            tc.tile_pool(name="cbuf", bufs=1) as cb,
            tc.tile_pool(name="prodb", bufs=3) as pb,
            tc.tile_pool(name="cpy", bufs=2, space="PSUM") as psY,
            tc.tile_pool(name="cpsc", bufs=2, space="PSUM") as psS,
            tc.tile_pool(name="dram", bufs=1, space="DRAM") as dp,
        ):
            # ---------------- DRAM scratch ----------------
            T_loc = dp.tile([EPC, P], F16)
            T_full = dp.tile([EPAD, P], F16, addr_space="Shared")
            W_loc = dp.tile([P * RSLOT, P], F16)   # row = j*RSLOT + slot

            # ---------------- inputs ----------------
            ang = cp.tile([P, ETILES, 72], F32)
            nc.sync.dma_start(out=ang[:], in_=ent[:].rearrange("t p k -> p t k"))
            wcoefb = cp.tile([P, 18, 7, WT], F16)
            nc.sync.dma_start(out=wcoefb[:], in_=wcoef_d[:])
            facW = cp.tile([P, 6, 6, WT], F16)
            nc.sync.dma_start(out=facW[:], in_=wfac_d[:])
            sidx = cp.tile([P, NIDX_T // 16], I16)
            nc.sync.dma_start(out=sidx[:], in_=sidx_d[:])
            oidx = cp.tile([P, NIDX_T // 16], I16)
            nc.sync.dma_start(out=oidx[:], in_=oidx_d[:])

            ones = cp.tile([P, 1], F16)
            nc.vector.memset(ones[:], 1.0)

            # const APs for activation scale/bias floats
            cdb = cp.tile([P, 3], F32)
            nc.vector.memset(cdb[:, 0:1], 0.0)
            nc.vector.memset(cdb[:, 1:2], 0.5)
            nc.vector.memset(cdb[:, 2:3], PI / 2)
            nc.const_aps.aps[(F32, 0.0)] = cdb[:, 0:1]
            nc.const_aps.aps[(F32, 0.5)] = cdb[:, 1:2]
            nc.const_aps.aps[(F32, PI / 2)] = cdb[:, 2:3]

            # ---------------- A: entity angle prep ----------------
            TA = ETILES
            angT = cp.tile([P, 72, TA], F32)     # tile-minor angles
            nc.vector.tensor_copy(
                out=angT[:], in_=ang[:].rearrange("p t k -> p k t"))
            gv = angT[:].rearrange("p (g a) t -> p g a t", g=24, a=3)
            phi, tha, omg = gv[:, :, 0, :], gv[:, :, 1, :], gv[:, :, 2, :]
            s1 = cp.tile([P, 24, TA], F32)
            s2 = cp.tile([P, 24, TA], F32)
            nc.vector.tensor_tensor(out=s1[:], in0=phi, in1=omg, op=ALU.add)
            nc.vector.tensor_tensor(out=s2[:], in0=phi, in1=omg, op=ALU.subtract)

            half = cp.tile([P, 6, 24, TA], F32)
            trig = cp.tile([P, 6, 24, TA], F32)  # ch sh ca sa cb sb
            hv, tv = half[:], trig[:]
            for i, srcv in ((0, tha), (2, s1[:]), (4, s2[:])):
                nc.vector.tensor_scalar(
                    out=hv[:, i], in0=srcv, scalar1=0.5, scalar2=PI / 2,
                    op0=ALU.mult, op1=ALU.add)
                nc.vector.tensor_scalar_mul(hv[:, i + 1], srcv, 0.5)
            for i in range(6):
                nc.scalar.activation(out=tv[:, i], in_=hv[:, i], func=ACTFN.Sin)

            # products v0..v3 for all 24 gates, f32
            vprod = cp.tile([P, 4, 24, TA], F32)
            vv = vprod[:]
            nc.vector.tensor_tensor(out=vv[:, 0], in0=tv[:, 0], in1=tv[:, 2], op=ALU.mult)
            nc.vector.tensor_tensor(out=vv[:, 1], in0=tv[:, 0], in1=tv[:, 3], op=ALU.mult)
            nc.vector.tensor_tensor(out=vv[:, 2], in0=tv[:, 1], in1=tv[:, 4], op=ALU.mult)
            nc.vector.tensor_tensor(out=vv[:, 3], in0=tv[:, 1], in1=tv[:, 5], op=ALU.mult)

            # base CRot coef table [p, 18, 7, T] fp16 (chain gates 6..23)
            abase = cp.tile([P, 18, 7, TA], F16)
            ab = abase[:]
            for k in range(4):
                nc.vector.tensor_copy(out=ab[:, :, k, :], in_=vv[:, k, 6:24, :])
            for dst, src in ((4, 1), (5, 2), (6, 3)):
                nc.vector.tensor_scalar_mul(ab[:, :, dst, :], vv[:, src, 6:24, :], -1.0)

            # layer-0 doubling factors facA [p, 6 steps, 6 slots, T]
            # step k expands wire q=5-k -> uses layer-0 gate qg=5-k
            # f0r=(v0-v2)r2  f0i=-(v1+v3)r2  f1r=(v0+v2)r2  f1i=(v1-v3)r2
            facA = cp.tile([P, 6, 6, TA], F16)
            t6 = cp.tile([P, 6, TA], F32)
            t6v = t6[:]
            fav = facA[:]

            def fac_dst(slot):
                # k dim reversed: k = 5 - g  (g = gate 0..5)
                base = fav.offset + 5 * (6 * TA) + slot * TA
                return bass.AP(tensor=fav.tensor, offset=base,
                               ap=[list(fav.ap[0]), [-6 * TA, 6], [1, TA]])
            g03 = vv[:, 0, 0:6, :], vv[:, 1, 0:6, :], vv[:, 2, 0:6, :], vv[:, 3, 0:6, :]
            v0g, v1g, v2g, v3g = g03
            nc.vector.tensor_tensor(out=t6v, in0=v0g, in1=v2g, op=ALU.subtract)
            nc.vector.tensor_scalar_mul(fac_dst(0), t6v, R2)
            nc.vector.tensor_tensor(out=t6v, in0=v1g, in1=v3g, op=ALU.add)
            nc.vector.tensor_scalar_mul(fac_dst(1), t6v, -R2)
            nc.vector.tensor_scalar_mul(fac_dst(2), t6v, R2)
            nc.vector.tensor_tensor(out=t6v, in0=v0g, in1=v2g, op=ALU.add)
            nc.vector.tensor_scalar_mul(fac_dst(3), t6v, R2)
            nc.vector.tensor_tensor(out=t6v, in0=v1g, in1=v3g, op=ALU.subtract)
            nc.vector.tensor_scalar_mul(fac_dst(4), t6v, R2)
            nc.vector.tensor_scalar_mul(fac_dst(5), t6v, -R2)

            # replicated coef tables
            acoef = cp.tile([P, NREP, TA], F16)
            _emit_coef_replicate(nc, abase, acoef, TA)
            wcoef = cp.tile([P, NREP, WT], F16)
            _emit_coef_replicate(nc, wcoefb, wcoef, WT)

            if stop_after == "prep":
                nc.sync.dma_start(out=dbg_d[:, 0:NREP * TA],
                                  in_=acoef[:].rearrange("p a t -> p (a t)"))
                nc.sync.dma_start(out=dbg_d[:, NREP * TA:NREP * TA + 360],
                                  in_=facA[:].rearrange("p a b t -> p (a b t)"))
            # ---------------- A chain ----------------
            if on("dblA"):
                sA0 = sp.tile([P, P, TA], F16)
                sA1 = sp.tile([P, P, TA], F16)
                cur = _emit_doubling(nc, gp, (sA0, sA1), facA[:], TA, "adbl")
                nxt = sA1
            if stop_after == "dblA":
                nc.sync.dma_start(out=dbg_d[:, 0:P * TA],
                                  in_=cur[:].rearrange("p a t -> p (a t)"))
            if on("chainA"):
                for g in range(18):
                    _emit_crot(nc, gp, cur, nxt, acoef, g, TA, "acr")
                    cur, nxt = nxt, cur
            if stop_after == "chainA":
                nc.sync.dma_start(out=dbg_d[:, 0:P * TA],
                                  in_=cur[:].rearrange("p a t -> p (a t)"))
            if on("storeT"):
                # T store (transpose to row-major) + AllGather
                Tout = cp.tile([P, TA, P], F16)
                nc.scalar.activation(
                    out=Tout[:], in_=cur[:].rearrange("p a t -> p t a"),
                    func=ACTFN.Copy)
                nc.sync.dma_start(
                    out=T_loc[:].rearrange("(t p) k -> p t k", p=P), in_=Tout[:])
                if no_collective:
                    nc.sync.dma_start(out=T_full[0:EPC, :], in_=T_loc[:])
                else:
                    nc.gpsimd.collective_compute(
                        "AllGather", ALU.bypass,
                        ins=[T_loc[:]], outs=[T_full[:]],
                        replica_groups=[list(range(NCORES))],
                    )

            if stop_after == "storeT":
                nc.sync.dma_start(out=dbg_d[:, 0:TA * P],
                                  in_=Tout[:].rearrange("p a t -> p (a t)"))

            # ---------------- W chain ----------------
            if on("chainW"):
                sW0 = sp.tile([P, P, WT], F16)
                sW1 = sp.tile([P, P, WT], F16)
                curw = _emit_doubling(nc, gp, (sW0, sW1), facW[:], WT, "wdbl")
                nxtw = sW1
                for g in range(18):
                    _emit_crot(nc, gp, curw, nxtw, wcoef, g, WT, "wcr")
                    curw, nxtw = nxtw, curw
            if stop_after == "chainW":
                nc.sync.dma_start(out=dbg_d[:, 0:P * WT],
                                  in_=curw[:].rearrange("p a t -> p (a t)"))

            # expand to W^T rows.  state partition p=(h,j): column j of W for
            # slot 2t+h, values [yr(64)|yi(64)].  W^T row j = state row;
            # W^T row 64+j = [-yi | yr].
            if on("storeW"):
                Wt = cp.tile([P, WT, P], F16)
                nc.scalar.activation(
                    out=Wt[:], in_=curw[:].rearrange("p a t -> p t a"),
                    func=ACTFN.Copy)
                Bt = cp.tile([P, WT, P], F16)
                nc.vector.tensor_scalar_mul(Bt[:, :, 0:64], Wt[:, :, 64:128], -1.0)
                nc.vector.tensor_copy(out=Bt[:, :, 64:128], in_=Wt[:, :, 0:64])
                # W_loc row = j*RSLOT + (2t+h);  p = h*64+j
                wl = W_loc[:].rearrange("(j t2 h) i -> h j t2 i", j=P, t2=WT, h=2)
                for h in (0, 1):
                    nc.sync.dma_start(out=wl[h, 0:64], in_=Wt[64 * h:64 * h + 64])
                    nc.sync.dma_start(out=wl[h, 64:128], in_=Bt[64 * h:64 * h + 64])
                # static supertile<->slot schedule (st == slot): whole W table
                # back to SBUF, no per-supertile gather needed.
                W_all = cp.tile([P, RSLOT, P], F16)
                nc.sync.dma_start(
                    out=W_all[:],
                    in_=W_loc[:].rearrange("(j s) i -> j s i", s=RSLOT))


            # ---------------- phase C gathers (chunked; SWDGE ring caps
            # one gather at ~896 idxs) ----------------
            GCH = 896
            qn = [0]

            def gather_chunks(out3, idxs_t, src, nidx, transpose):
                s = 0
                while s < nidx:
                    n = min(GCH, nidx - s)
                    if transpose:
                        o = out3[:, :, s:s + n]
                    else:
                        o = out3[:, s // P:(s + n) // P, :]
                    nc.gpsimd.dma_gather(
                        out_ap=o, in_ap=src, idxs_ap=idxs_t[:, s // 16:(s + n) // 16],
                        num_idxs=n, num_idxs_reg=n, elem_size=P,
                        transpose=transpose, queue_num=0)
                    qn[0] = 0
                    s += n

            SUBS = [7, 7, 7, 5]                # supertiles per subtile
            if on("gather"):
              TsTs, ToTs = [], []
              for si, nst in enumerate(SUBS):
                  tsub = cb.tile([P, 1, nst * STW], F16, tag=f"tst{si}", name=f"TsT{si}")
                  osub = cb.tile([P, 1, nst * STW], F16, tag=f"tot{si}", name=f"ToT{si}")
                  TsTs.append(tsub)
                  ToTs.append(osub)
              base = 0
              for si, nst in enumerate(SUBS):
                  nid = nst * STW
                  s = 0
                  while s < nid:
                      n = min(GCH, nid - s)
                      for (buf, idxs_t) in ((TsTs[si], sidx), (ToTs[si], oidx)):
                          nc.gpsimd.dma_gather(
                              out_ap=buf[:, :, s:s + n], in_ap=T_full[:],
                              idxs_ap=idxs_t[:, (base + s) // 16:(base + s + n) // 16],
                              num_idxs=n, num_idxs_reg=n, elem_size=P,
                              transpose=True, queue_num=0)
                      s += n
                  base += nid
            if stop_after == "gather":
                nc.sync.dma_start(out=dbg_d[:, 0:2048],
                                  in_=TsTs[0][:, 0, 0:2048])
            if stop_after == "gatherw":
                nc.sync.dma_start(out=dbg_d[:, 0:3328],
                                  in_=W_all[:].rearrange("p a b -> p (a b)"))

            if on("full"):
              scores = cp.tile([1, NST * STW], F32)
              st = 0
              for si, nst in enumerate(SUBS):
                for li in range(nst):
                 pY = psY.tile([P, STW], F32, tag="py")
                 nc.tensor.matmul(out=pY[:], lhsT=W_all[:, st, :],
                                  rhs=TsTs[si][:, 0, li * STW:(li + 1) * STW],
                                  start=True, stop=True)
                 prod = pb.tile([P, STW], F16, tag="prod")
                 nc.vector.tensor_tensor(
                     out=prod[:], in0=pY[:],
                     in1=ToTs[si][:, 0, li * STW:(li + 1) * STW], op=ALU.mult)
                 pS = psS.tile([1, STW], F32, tag="ps")
                 nc.tensor.matmul(out=pS[:], lhsT=ones[:], rhs=prod[:],
                                  start=True, stop=True)
                 nc.scalar.activation(
                     out=scores[0:1, st * STW:(st + 1) * STW], in_=pS[:],
                     func=ACTFN.Copy)
                 st += 1
              nc.sync.dma_start(out=scores_d[:], in_=scores[:])

    nc.finalize()
    return nc


# --------------------------------------------------------------------------
# host side
# --------------------------------------------------------------------------

def _rot_elems(params):
    """params [..., 3] (phi, theta, omega) -> v0..v3 f32.

    m00=(v0,-v1) m01=(-v2,-v3) m10=(v2,-v3) m11=(v0,v1)
    """
    phi, tha, omg = params[..., 0], params[..., 1], params[..., 2]
    ch, sh = np.cos(tha / 2), np.sin(tha / 2)
    a, b = (phi + omg) / 2, (phi - omg) / 2
    return (
        (ch * np.cos(a)).astype(np.float32),
        (ch * np.sin(a)).astype(np.float32),
        (sh * np.cos(b)).astype(np.float32),
        (sh * np.sin(b)).astype(np.float32),
    )


def _pack_idxs(flat):
    """int array [n] (n % 16 == 0) -> [128, n/16] int16 (replicated x8)."""
    n = len(flat)
    blk = np.asarray(flat, np.int16).reshape(n // 16, 16).T
    return np.tile(blk, (8, 1))


def _host_prep(entity_params, relation_params, s_idx, p_idx, o_idx):
    ent = np.asarray(entity_params, dtype=np.float32)
    rel = np.asarray(relation_params, dtype=np.float32)
    s_idx = np.asarray(s_idx)
    p_idx = np.asarray(p_idx)
    o_idx = np.asarray(o_idx)

    # ---- entity shards ----
    ent_flat = ent.reshape(E, 72)
    ent_pad = np.zeros((EPAD, 72), np.float32)
    ent_pad[:E] = ent_flat
    ent_shards = [
        ent_pad[c * EPC:(c + 1) * EPC].reshape(ETILES, P, 72)
        for c in range(NCORES)
    ]

    # ---- supertiles (relation, <=512 elements) ----
    order = np.argsort(p_idx, kind="stable")
    bounds = np.searchsorted(p_idx[order], np.arange(R + 1))
    tiles = []
    for r in range(R):
        grp = order[bounds[r]:bounds[r + 1]]
        for i in range(0, len(grp), STW):
            tiles.append((r, grp[i:i + STW]))
    ntiles = len(tiles)
    assert ntiles <= NCORES * NST, f"too many supertiles {ntiles}"

    # greedy balance: relations (grouped) to cores, bounded slots+tiles
    by_rel = {}
    for t in tiles:
        by_rel.setdefault(t[0], []).append(t)
    core_tiles = [[] for _ in range(NCORES)]
    for r, ts in sorted(by_rel.items(), key=lambda kv: -len(kv[1])):
        remaining = list(ts)
        while remaining:
            c = min(range(NCORES), key=lambda cc: len(core_tiles[cc]))
            cap = NST - len(core_tiles[c])
            assert cap > 0, "balance failure"
            take = remaining[:cap]
            core_tiles[c].extend(take)
            remaining = remaining[len(take):]

    gate_rel = rel.reshape(R, 24, 3)
    v0, v1, v2, v3 = _rot_elems(gate_rel)   # each [R, 24]

    in_maps = []
    outpos = np.full((NCORES, NST, STW), -1, np.int64)
    for c in range(NCORES):
        ct = core_tiles[c]
        assert len(ct) <= NST
        # static schedule: slot st computes supertile st's relation
        slot_rel = [ct[t][0] if t < len(ct) else 0 for t in range(RSLOT)]

        sflat = np.zeros(NIDX_T, np.int64)
        oflat = np.zeros(NIDX_T, np.int64)
        for t in range(NST):
            if t < len(ct):
                r, elems = ct[t]
                n = len(elems)
                sflat[t * STW:t * STW + n] = s_idx[elems]
                oflat[t * STW:t * STW + n] = o_idx[elems]
                outpos[c, t, :n] = elems

        # W-chain coef [p, 18, 7, WT] fp16 + doubling factors [p,6,6,WT]
        wcoefb = np.zeros((P, 18, 7, WT), np.float16)
        wfac = np.zeros((P, 6, 6, WT), np.float16)
        for sl, r in enumerate(slot_rel):
            wt, hh = divmod(sl, 2)
            rows = slice(64 * hh, 64 * hh + 64)
            for g in range(18):
                gg = 6 + g
                vals = (v0[r, gg], v1[r, gg], v2[r, gg], v3[r, gg],
                        -v1[r, gg], -v2[r, gg], -v3[r, gg])
                for k, vvv in enumerate(vals):
                    wcoefb[rows, g, k, wt] = vvv
            j = np.arange(64)
            for k in range(6):
                qg = 5 - k
                bit = (j >> k) & 1
                m00 = (v0[r, qg], -v1[r, qg])
                m01 = (-v2[r, qg], -v3[r, qg])
                m10 = (v2[r, qg], -v3[r, qg])
                m11 = (v0[r, qg], v1[r, qg])
                u0r = np.where(bit == 0, m00[0], m01[0])
                u0i = np.where(bit == 0, m00[1], m01[1])
                u1r = np.where(bit == 0, m10[0], m11[0])
                u1i = np.where(bit == 0, m10[1], m11[1])
                wfac[rows, k, 0, wt] = u0r
                wfac[rows, k, 1, wt] = u0i
                wfac[rows, k, 2, wt] = -u0i
                wfac[rows, k, 3, wt] = u1r
                wfac[rows, k, 4, wt] = u1i
                wfac[rows, k, 5, wt] = -u1i

        in_maps.append({
            "ent_par": ent_shards[c],
            "wcoefb": wcoefb,
            "wfac": wfac,
            "sidx": _pack_idxs(sflat),
            "oidx": _pack_idxs(oflat),
        })
    return in_maps, outpos


_PROGRAM = None


def kernel(entity_params, relation_params, s_idx, p_idx, o_idx):
    global _PROGRAM
    in_maps, outpos = _host_prep(entity_params, relation_params,
                                 s_idx, p_idx, o_idx)
    if _PROGRAM is None:
        _PROGRAM = build_program()
    nc = _PROGRAM
    res = run_bass_kernel_spmd(nc, in_maps, list(range(NCORES)))
    out = np.zeros(B, np.float32)
    for c in range(NCORES):
        sc = res.results[c]["scores"].reshape(NST, STW)
        pos = outpos[c]
        mask = pos >= 0
        out[pos[mask]] = sc[mask]
    return out


if __name__ == "__main__":
    build_program()
    print("build OK")


# revision 25
# speedup vs baseline: 1.0525x; 1.0121x over previous
"""Trainium2 Bass kernel for the quantum-circuit KG-embedding scoring model.

score(s,p,o) = Re(<B_o h | W_p | B_s h>), B_e / W_p = 24-gate circuit blocks,
h = |+>^6.  State dim 64 complex = 128 reals [re(64) | im(64)].

Device algorithm (8 cores, SPMD), fp16 throughout the heavy paths:

  Chain phase (DVE, fp16 tile-minor layout [128 ent, 128 amp-real, T]):
    A-chain: evolve 1280 entities/core (10 tiles innermost) through the
    entity block (product-state doubling + 18 CRots).  Every DVE op has a
    stride-1 fp16 innermost dim -> 2x perf mode; coefficient tables are
    pre-replicated over the gate's low amp bits so no op exceeds 3 free
    dims (TENSOR3D) and no gate needs group splitting.
    Store T rows fp16 -> AllGather -> T_full [10240,128] fp16.
    W-chain: same machinery on 13 tiles = 26 relation slots x 64 basis
    columns, overlapping the AllGather.  Expand to W^T rows in DRAM.

  Phase C (supertiles of 512 elements, one relation each):
    dma_gather(transpose=True) pulls Ts^T / To^T [128 state, 13312 elems]
    straight from T_full (0.34ns/descriptor on gpsimd vs ~12ns for
    per-row indirect DMA; transfers spread over 16 DMA engines).
    dma_gather pulls per-supertile W^T [128,128] tiles.
    Per supertile: PE fp16 matmul Y = W @ Ts^T (psum f32), DVE
    prod = Y * To^T (fp16), PE ones-matmul column-sum -> scores.

Host does only: trig for the 200-relation tables, index sort/packing,
output unpermute (same division of labour as the reference baseline).
"""

import sys
import numpy as np

for _p in ("/opt/trn_rl_repo",):
    if _p not in sys.path:
        sys.path.insert(0, _p)

import concourse.bass as bass
import concourse.bacc as bacc
import concourse.mybir as mybir
from concourse import tile
from concourse.bass_utils import run_bass_kernel_spmd

F32 = mybir.dt.float32
F16 = mybir.dt.float16
I16 = mybir.dt.int16
ALU = mybir.AluOpType
ACTFN = mybir.ActivationFunctionType

P = 128
Q = 6
NA = 64                      # 2^Q amplitudes
NCORES = 8
E, R, B = 10000, 200, 65536
ETILES = 10                  # entity tiles per core
EPC = ETILES * P             # 1280 entities per core
EPAD = EPC * NCORES          # 10240 padded entity rows
WT = 13                      # W-chain tiles per core (2 rel slots each)
RSLOT = 2 * WT               # 26 relation slots per core
NST = 26                     # phase-C supertiles per core
STW = 512                    # supertile width (elements)
NIDX_T = NST * STW           # 13312 gathered T rows per table per core
NIDX_W = NST * P             # 3328 gathered W^T rows per core
R2 = float(2.0 ** -0.5)
PI = float(np.pi)

# CRot gate list: (control, target) wire pairs, in circuit order
CROTS = [(q, (q + off) % Q) for off in (1, 2, 3) for q in range(Q)]

# per-gate geometry: amp bit positions cpos=5-c (control), tpos=5-t
_GEO = []
for (c, t) in CROTS:
    cpos, tpos = 5 - c, 5 - t
    hi, lo = max(cpos, tpos), min(cpos, tpos)
    A = 1 << (5 - hi)
    Bm = 1 << (hi - lo - 1)
    C = 1 << lo
    _GEO.append((cpos, tpos, hi, lo, A, Bm, C))

# replicated coefficient table layout: per gate, 7 slots each replicated C
# times: block [7, C, T] at offset OFF[g] (in slot-columns of width T)
_OFF = []
_o = 0
for (_, _, _, _, _, _, C) in _GEO:
    _OFF.append(_o)
    _o += 7 * C
NREP = _o                    # total replicated slot-columns (= 7 * sum C)

# coefficient slot roles: 0:v0 1:v1 2:v2 3:v3 4:-v1 5:-v2 6:-v3
# output quarter <- sum of (slot, input quarter):
COMP_TERMS = {
    "a0r": [(0, "a0r"), (1, "a0i"), (5, "a1r"), (3, "a1i")],
    "a0i": [(4, "a0r"), (0, "a0i"), (6, "a1r"), (5, "a1i")],
    "a1r": [(2, "a0r"), (3, "a0i"), (0, "a1r"), (4, "a1i")],
    "a1i": [(6, "a0r"), (2, "a0i"), (1, "a1r"), (0, "a1i")],
}
QKEYS = {"a0r": (0, 0), "a0i": (1, 0), "a1r": (0, 1), "a1i": (1, 1)}


def _mk_ap(src_ap, dims):
    """Manual AP with explicit [stride, count] dims (partition dim first)."""
    return bass.AP(tensor=src_ap.tensor, offset=src_ap.offset,
                   ap=[list(d) for d in dims])


def _quarter_aps(st_ap, g, T):
    """Quarter-slice APs of state [128, 128, T] for CRot gate g.

    Returns dict name -> AP with dims [p, (A?), (Bm?), C*T] (c merged with
    the innermost tile dim; requires the state tile's last dim == T exactly).
    """
    cpos, tpos, hi, lo, A, Bm, C = _GEO[g]
    cbit_is_x = (cpos == hi)
    pdim = list(st_ap.ap[0])
    base_off = st_ap.offset
    # element strides within one partition (state tile is [128, 128, T],
    # contiguous): amp stride = T, tile stride = 1
    s_r = 64 * T
    s_a = 2 * Bm * 2 * C * T
    s_x = Bm * 2 * C * T
    s_b = 2 * C * T
    s_y = C * T
    out = {}
    for name, (r, tval) in QKEYS.items():
        xbit, ybit = (1, tval) if cbit_is_x else (tval, 1)
        off = base_off + r * s_r + xbit * s_x + ybit * s_y
        dims = [pdim]
        if A > 1:
            dims.append([s_a, A])
        if Bm > 1:
            dims.append([s_b, Bm])
        dims.append([1, C * T])
        out[name] = _mk_ap(st_ap, dims)
        out[name] = bass.AP(tensor=st_ap.tensor, offset=off,
                            ap=out[name].ap)
    return out


def _coef_aps(coef_ap, g, T):
    """Slot APs (broadcast to quarter shape) from replicated coef table
    [128, NREP, T].  Slot k of gate g occupies columns OFF[g]+k*C ..
    +C, real memory, so the merged (C*T) innermost dim is stride-1."""
    cpos, tpos, hi, lo, A, Bm, C = _GEO[g]
    pdim = list(coef_ap.ap[0])
    out = []
    for k in range(7):
        off = coef_ap.offset + (_OFF[g] + k * C) * T
        dims = [pdim]
        if A > 1:
            dims.append([0, A])
        if Bm > 1:
            dims.append([0, Bm])
        dims.append([1, C * T])
        out.append(bass.AP(tensor=coef_ap.tensor, offset=off, ap=dims))
    return out


def _emit_crot(nc, pool, mpool, cur, nxt, coef, g, T, tag, offload=False):
    """One CRot gate: read cur, write nxt (ping-pong), fp16 tile-minor.

    offload=True runs one of the four outputs on gpsimd (useful while
    gpsimd would otherwise idle, e.g. during the A chain)."""
    cpos, tpos, hi, lo, A, Bm, C = _GEO[g]
    qc = _quarter_aps(cur[:], g, T)
    qn = _quarter_aps(nxt[:], g, T)
    co = _coef_aps(coef[:], g, T)
    fsz = A * Bm * C * T                       # quarter free size
    names = list(COMP_TERMS)
    mblk = mpool.tile([P, 16 * fsz], F16, tag=f"{tag}mb", name="mblk")
    mb = mblk[:]

    def slot_flat(k):
        return mb[:, k * fsz:(k + 1) * fsz]

    def slot_shaped(k, like):
        dims = [list(mb.ap[0])]
        stride = 1
        rev = []
        for d in reversed(like.ap[1:]):
            rev.append([stride, d[1]])
            stride *= d[1]
        dims += rev[::-1]
        return bass.AP(tensor=mb.tensor, offset=mb.offset + k * fsz, ap=dims)
    # 16 independent multiplies first (no RAW stalls on DVE) ...
    for oi, name in enumerate(names):
        for ti, (slot, inp) in enumerate(COMP_TERMS[name]):
            nc.vector.tensor_tensor(out=slot_shaped(oi * 4 + ti, qc[inp]),
                                    in0=qc[inp], in1=co[slot], op=ALU.mult)
    # ... then pair-adds on flat contiguous slices (2x eligible) ...
    for oi in range(4):
        nc.vector.tensor_tensor(out=slot_flat(oi * 4), in0=slot_flat(oi * 4),
                                in1=slot_flat(oi * 4 + 1), op=ALU.add)
        nc.vector.tensor_tensor(out=slot_flat(oi * 4 + 2),
                                in0=slot_flat(oi * 4 + 2),
                                in1=slot_flat(oi * 4 + 3), op=ALU.add)
    # ... then the final adds into the next state buffer
    for oi, name in enumerate(names):
        nc.vector.tensor_tensor(out=qn[name],
                                in0=slot_shaped(oi * 4, qn[name]),
                                in1=slot_shaped(oi * 4 + 2, qn[name]),
                                op=ALU.add)
    # inactive (control=0) half: single merged copy cur -> nxt on ACT
    u = 64 >> cpos                              # dims above cpos incl r
    m = 1 << cpos
    s_c = m * T
    for st_ap, dst in ((cur[:], 0), (nxt[:], 1)):
        dims = [list(st_ap.ap[0])]
        if u > 1:
            dims.append([2 * s_c, u])
        dims.append([1, m * T])
        ap = bass.AP(tensor=st_ap.tensor, offset=st_ap.offset, ap=dims)
        if dst == 0:
            src_ap = ap
        else:
            dst_ap = ap
    nc.scalar.activation(out=dst_ap, in_=src_ap, func=ACTFN.Copy)


def _emit_doubling(nc, pool, sbufs, fac, T, tag):
    """Product state from factors, tile-minor.

    sbufs: (sA, sB) [128, 128, T] fp16.  fac: [128, 6, 6, T] fp16 with
    per-step slots [u0r, u0i, -u0i, u1r, u1i, -u1i]; step k expands wire
    q=5-k.  Returns the buffer holding the result (sA).
    """
    sA, sB = sbufs
    cur = sA
    for (dst_col, src_slot) in ((0, 0), (1, 3)):       # re: u0r, u1r
        nc.vector.tensor_copy(out=cur[:, dst_col, :],
                              in_=fac[:, 0, src_slot, :])
    for (dst_col, src_slot) in ((64, 1), (65, 4)):     # im: u0i, u1i
        nc.vector.tensor_copy(out=cur[:, dst_col, :],
                              in_=fac[:, 0, src_slot, :])
    for k in range(1, 6):
        w = 1 << k
        nxt = sB if cur is sA else sA
        fv = fac[:]

        def fpair(slot):
            # slots (slot, slot+3) for m=0/1: [p, m:2, w(bcast), T]
            off = fv.offset + (k * 6 + slot) * T
            return bass.AP(tensor=fv.tensor, offset=off,
                           ap=[list(fv.ap[0]), [3 * T, 2], [0, w], [1, T]])

        def mview(ap_base, col0):
            # [p, m:2, w, T] over state cols [col0, col0+2w)
            off = ap_base.offset + col0 * T
            return bass.AP(tensor=ap_base.tensor, offset=off,
                           ap=[list(ap_base.ap[0]), [w * T, 2], [T, w], [1, T]])

        def cbc(ap_base, col0):
            # [p, m-bcast:2, w, T] over cur cols [col0, col0+w)
            off = ap_base.offset + col0 * T
            return bass.AP(tensor=ap_base.tensor, offset=off,
                           ap=[list(ap_base.ap[0]), [0, 2], [T, w], [1, T]])
        crb, cib = cbc(cur[:], 0), cbc(cur[:], 64)
        dr, di = mview(nxt[:], 0), mview(nxt[:], 64)
        t1 = pool.tile([P, 2 * w * T], F16, tag=tag + "a")
        t2 = pool.tile([P, 2 * w * T], F16, tag=tag + "b")
        t1v = t1[:].rearrange("p (m w t) -> p m w t", m=2, w=w)
        t2v = t2[:].rearrange("p (m w t) -> p m w t", m=2, w=w)
        nc.vector.tensor_tensor(out=t1v, in0=crb, in1=fpair(0), op=ALU.mult)
        nc.vector.tensor_tensor(out=t2v, in0=cib, in1=fpair(2), op=ALU.mult)
        nc.vector.tensor_tensor(out=dr, in0=t1v, in1=t2v, op=ALU.add)
        nc.vector.tensor_tensor(out=t1v, in0=crb, in1=fpair(1), op=ALU.mult)
        nc.vector.tensor_tensor(out=t2v, in0=cib, in1=fpair(0), op=ALU.mult)
        nc.vector.tensor_tensor(out=di, in0=t1v, in1=t2v, op=ALU.add)
        cur = nxt
    if cur is not sA:
        nc.vector.tensor_copy(out=sA[:], in_=cur[:])
    return sA


def _emit_coef_replicate(nc, base, coef, T):
    """base [128, 18, 7, T] -> replicated coef [128, NREP, T]."""
    cv = coef[:]
    for g in range(18):
        C = _GEO[g][6]
        src = base[:, g, :, :].unsqueeze(2).to_broadcast([P, 7, C, T])
        dst = bass.AP(tensor=cv.tensor, offset=cv.offset + _OFF[g] * T,
                      ap=[list(cv.ap[0]), [C * T, 7], [T, C], [1, T]])
        nc.vector.tensor_copy(out=dst, in_=src)


_STAGES = ["prep", "dblA", "chainA", "storeT", "chainW", "storeW", "gather", "gatherw", "full"]


def build_program(stop_after=None, no_collective=False):
    lim = _STAGES.index(stop_after) if stop_after else len(_STAGES)

    def on(stage):
        return _STAGES.index(stage) < lim or stage == stop_after
    nc = bacc.Bacc("TRN2", target_bir_lowering=False, debug=False,
                   num_swdge_queues=4)

    ent = nc.dram_tensor("ent_par", [ETILES, P, 72], F32, kind="ExternalInput")
    wcoef_d = nc.dram_tensor("wcoefb", [P, 18, 7, WT], F16, kind="ExternalInput")
    wfac_d = nc.dram_tensor("wfac", [P, 6, 6, WT], F16, kind="ExternalInput")
    sidx_d = nc.dram_tensor("sidx", [P, NIDX_T // 16], I16, kind="ExternalInput")
    oidx_d = nc.dram_tensor("oidx", [P, NIDX_T // 16], I16, kind="ExternalInput")
    scores_d = nc.dram_tensor("scores", [1, NST * STW], F16, kind="ExternalOutput")
    dbg_d = nc.dram_tensor("dbg", [P, 8192], F16, kind="ExternalOutput") if stop_after else None

    with tile.TileContext(nc) as tc:
        with (
            tc.tile_pool(name="const", bufs=1) as cp,
            tc.tile_pool(name="gtmp", bufs=2) as gp,
            tc.tile_pool(name="mbp", bufs=1) as mbp,
            tc.tile_pool(name="state", bufs=1) as sp,
            tc.tile_pool(name="cbuf", bufs=1) as cb,
            tc.tile_pool(name="prodb", bufs=3) as pb,
            tc.tile_pool(name="cpy", bufs=2, space="PSUM") as psY,
            tc.tile_pool(name="cpsc", bufs=2, space="PSUM") as psS,
            tc.tile_pool(name="dram", bufs=1, space="DRAM") as dp,
        ):
            # ---------------- DRAM scratch ----------------
            T_loc = dp.tile([EPC, P], F16)
            T_full = dp.tile([EPAD, P], F16, addr_space="Shared")
            W_loc = dp.tile([P * RSLOT, P], F16)   # row = j*RSLOT + slot

            # ---------------- inputs ----------------
            ang = cp.tile([P, ETILES, 72], F32)
            nc.sync.dma_start(out=ang[:], in_=ent[:].rearrange("t p k -> p t k"))
            wcoefb = cp.tile([P, 18, 7, WT], F16)
            nc.sync.dma_start(out=wcoefb[:], in_=wcoef_d[:])
            facW = cp.tile([P, 6, 6, WT], F16)
            nc.sync.dma_start(out=facW[:], in_=wfac_d[:])
            sidx = cp.tile([P, NIDX_T // 16], I16)
            nc.sync.dma_start(out=sidx[:], in_=sidx_d[:])
            oidx = cp.tile([P, NIDX_T // 16], I16)
            nc.sync.dma_start(out=oidx[:], in_=oidx_d[:])

            ones = cp.tile([P, 1], F16)
            nc.vector.memset(ones[:], 1.0)

            # const APs for activation scale/bias floats
            cdb = cp.tile([P, 3], F32)
            nc.vector.memset(cdb[:, 0:1], 0.0)
            nc.vector.memset(cdb[:, 1:2], 0.5)
            nc.vector.memset(cdb[:, 2:3], PI / 2)
            nc.const_aps.aps[(F32, 0.0)] = cdb[:, 0:1]
            nc.const_aps.aps[(F32, 0.5)] = cdb[:, 1:2]
            nc.const_aps.aps[(F32, PI / 2)] = cdb[:, 2:3]

            # ---------------- A: entity angle prep ----------------
            TA = ETILES
            angT = cp.tile([P, 72, TA], F32)     # tile-minor angles
            nc.vector.tensor_copy(
                out=angT[:], in_=ang[:].rearrange("p t k -> p k t"))
            gv = angT[:].rearrange("p (g a) t -> p g a t", g=24, a=3)
            phi, tha, omg = gv[:, :, 0, :], gv[:, :, 1, :], gv[:, :, 2, :]
            s1 = cp.tile([P, 24, TA], F32)
            s2 = cp.tile([P, 24, TA], F32)
            nc.vector.tensor_tensor(out=s1[:], in0=phi, in1=omg, op=ALU.add)
            nc.vector.tensor_tensor(out=s2[:], in0=phi, in1=omg, op=ALU.subtract)

            half = cp.tile([P, 6, 24, TA], F32)
            trig = cp.tile([P, 6, 24, TA], F32)  # ch sh ca sa cb sb
            hv, tv = half[:], trig[:]
            for i, srcv in ((0, tha), (2, s1[:]), (4, s2[:])):
                nc.vector.tensor_scalar(
                    out=hv[:, i], in0=srcv, scalar1=0.5, scalar2=PI / 2,
                    op0=ALU.mult, op1=ALU.add)
                nc.vector.tensor_scalar_mul(hv[:, i + 1], srcv, 0.5)
            for i in range(6):
                nc.scalar.activation(out=tv[:, i], in_=hv[:, i], func=ACTFN.Sin)

            # products v0..v3 for all 24 gates, f32
            vprod = cp.tile([P, 4, 24, TA], F32)
            vv = vprod[:]
            nc.vector.tensor_tensor(out=vv[:, 0], in0=tv[:, 0], in1=tv[:, 2], op=ALU.mult)
            nc.vector.tensor_tensor(out=vv[:, 1], in0=tv[:, 0], in1=tv[:, 3], op=ALU.mult)
            nc.vector.tensor_tensor(out=vv[:, 2], in0=tv[:, 1], in1=tv[:, 4], op=ALU.mult)
            nc.vector.tensor_tensor(out=vv[:, 3], in0=tv[:, 1], in1=tv[:, 5], op=ALU.mult)

            # base CRot coef table [p, 18, 7, T] fp16 (chain gates 6..23)
            abase = cp.tile([P, 18, 7, TA], F16)
            ab = abase[:]
            for k in range(4):
                nc.vector.tensor_copy(out=ab[:, :, k, :], in_=vv[:, k, 6:24, :])
            for dst, src in ((4, 1), (5, 2), (6, 3)):
                nc.vector.tensor_scalar_mul(ab[:, :, dst, :], vv[:, src, 6:24, :], -1.0)

            # layer-0 doubling factors facA [p, 6 steps, 6 slots, T]
            # step k expands wire q=5-k -> uses layer-0 gate qg=5-k
            # f0r=(v0-v2)r2  f0i=-(v1+v3)r2  f1r=(v0+v2)r2  f1i=(v1-v3)r2
            facA = cp.tile([P, 6, 6, TA], F16)
            t6 = cp.tile([P, 6, TA], F32)
            t6v = t6[:]
            fav = facA[:]

            def fac_dst(slot):
                # k dim reversed: k = 5 - g  (g = gate 0..5)
                base = fav.offset + 5 * (6 * TA) + slot * TA
                return bass.AP(tensor=fav.tensor, offset=base,
                               ap=[list(fav.ap[0]), [-6 * TA, 6], [1, TA]])
            g03 = vv[:, 0, 0:6, :], vv[:, 1, 0:6, :], vv[:, 2, 0:6, :], vv[:, 3, 0:6, :]
            v0g, v1g, v2g, v3g = g03
            nc.vector.tensor_tensor(out=t6v, in0=v0g, in1=v2g, op=ALU.subtract)
            nc.vector.tensor_scalar_mul(fac_dst(0), t6v, R2)
            nc.vector.tensor_tensor(out=t6v, in0=v1g, in1=v3g, op=ALU.add)
            nc.vector.tensor_scalar_mul(fac_dst(1), t6v, -R2)
            nc.vector.tensor_scalar_mul(fac_dst(2), t6v, R2)
            nc.vector.tensor_tensor(out=t6v, in0=v0g, in1=v2g, op=ALU.add)
            nc.vector.tensor_scalar_mul(fac_dst(3), t6v, R2)
            nc.vector.tensor_tensor(out=t6v, in0=v1g, in1=v3g, op=ALU.subtract)
            nc.vector.tensor_scalar_mul(fac_dst(4), t6v, R2)
            nc.vector.tensor_scalar_mul(fac_dst(5), t6v, -R2)

            # replicated coef tables
            acoef = cp.tile([P, NREP, TA], F16)
            _emit_coef_replicate(nc, abase, acoef, TA)
            wcoef = cp.tile([P, NREP, WT], F16)
            _emit_coef_replicate(nc, wcoefb, wcoef, WT)

            if stop_after == "prep":
                nc.sync.dma_start(out=dbg_d[:, 0:NREP * TA],
                                  in_=acoef[:].rearrange("p a t -> p (a t)"))
                nc.sync.dma_start(out=dbg_d[:, NREP * TA:NREP * TA + 360],
                                  in_=facA[:].rearrange("p a b t -> p (a b t)"))
            # ---------------- A chain ----------------
            if on("dblA"):
                sA0 = sp.tile([P, P, TA], F16)
                sA1 = sp.tile([P, P, TA], F16)
                cur = _emit_doubling(nc, gp, (sA0, sA1), facA[:], TA, "adbl")
                nxt = sA1
            if stop_after == "dblA":
                nc.sync.dma_start(out=dbg_d[:, 0:P * TA],
                                  in_=cur[:].rearrange("p a t -> p (a t)"))
            if on("chainA"):
                for g in range(18):
                    _emit_crot(nc, gp, mbp, cur, nxt, acoef, g, TA, "acr")
                    cur, nxt = nxt, cur
            if stop_after == "chainA":
                nc.sync.dma_start(out=dbg_d[:, 0:P * TA],
                                  in_=cur[:].rearrange("p a t -> p (a t)"))
            if on("storeT"):
                # T store (transpose to row-major) + AllGather
                Tout = cp.tile([P, TA, P], F16)
                nc.scalar.activation(
                    out=Tout[:], in_=cur[:].rearrange("p a t -> p t a"),
                    func=ACTFN.Copy)
                nc.sync.dma_start(
                    out=T_loc[:].rearrange("(t p) k -> p t k", p=P), in_=Tout[:])
                if no_collective:
                    nc.sync.dma_start(out=T_full[0:EPC, :], in_=T_loc[:])
                else:
                    nc.gpsimd.collective_compute(
                        "AllGather", ALU.bypass,
                        ins=[T_loc[:]], outs=[T_full[:]],
                        replica_groups=[list(range(NCORES))],
                    )

            if stop_after == "storeT":
                nc.sync.dma_start(out=dbg_d[:, 0:TA * P],
                                  in_=Tout[:].rearrange("p a t -> p (a t)"))

            # ---------------- W chain ----------------
            if on("chainW"):
                sW0 = sp.tile([P, P, WT], F16)
                sW1 = sp.tile([P, P, WT], F16)
                curw = _emit_doubling(nc, gp, (sW0, sW1), facW[:], WT, "wdbl")
                nxtw = sW1
                for g in range(18):
                    _emit_crot(nc, gp, mbp, curw, nxtw, wcoef, g, WT, "wcr")
                    curw, nxtw = nxtw, curw
            if stop_after == "chainW":
                nc.sync.dma_start(out=dbg_d[:, 0:P * WT],
                                  in_=curw[:].rearrange("p a t -> p (a t)"))

            # expand to W^T rows.  state partition p=(h,j): column j of W for
            # slot 2t+h, values [yr(64)|yi(64)].  W^T row j = state row;
            # W^T row 64+j = [-yi | yr].
            if on("storeW"):
                Wt = cp.tile([P, WT, P], F16)
                nc.scalar.activation(
                    out=Wt[:], in_=curw[:].rearrange("p a t -> p t a"),
                    func=ACTFN.Copy)
                Bt = cp.tile([P, WT, P], F16)
                nc.vector.tensor_scalar_mul(Bt[:, :, 0:64], Wt[:, :, 64:128], -1.0)
                nc.vector.tensor_copy(out=Bt[:, :, 64:128], in_=Wt[:, :, 0:64])
                # W_loc row = j*RSLOT + (2t+h);  p = h*64+j
                wl = W_loc[:].rearrange("(j t2 h) i -> h j t2 i", j=P, t2=WT, h=2)
                for h in (0, 1):
                    nc.sync.dma_start(out=wl[h, 0:64], in_=Wt[64 * h:64 * h + 64])
                    nc.sync.dma_start(out=wl[h, 64:128], in_=Bt[64 * h:64 * h + 64])
                # static supertile<->slot schedule (st == slot): whole W table
                # back to SBUF, no per-supertile gather needed.
                W_all = cp.tile([P, RSLOT, P], F16)
                nc.sync.dma_start(
                    out=W_all[:],
                    in_=W_loc[:].rearrange("(j s) i -> j s i", s=RSLOT))


            # ---------------- phase C gathers (chunked; SWDGE ring caps
            # one gather at ~896 idxs) ----------------
            GCH = 896
            qn = [0]

            def gather_chunks(out3, idxs_t, src, nidx, transpose):
                s = 0
                while s < nidx:
                    n = min(GCH, nidx - s)
                    if transpose:
                        o = out3[:, :, s:s + n]
                    else:
                        o = out3[:, s // P:(s + n) // P, :]
                    nc.gpsimd.dma_gather(
                        out_ap=o, in_ap=src, idxs_ap=idxs_t[:, s // 16:(s + n) // 16],
                        num_idxs=n, num_idxs_reg=n, elem_size=P,
                        transpose=transpose, queue_num=0)
                    qn[0] = 0
                    s += n

            SUBS = [7, 7, 7, 5]                # supertiles per subtile
            if on("gather"):
              TsTs, ToTs = [], []
              for si, nst in enumerate(SUBS):
                  tsub = cb.tile([P, 1, nst * STW], F16, tag=f"tst{si}", name=f"TsT{si}")
                  osub = cb.tile([P, 1, nst * STW], F16, tag=f"tot{si}", name=f"ToT{si}")
                  TsTs.append(tsub)
                  ToTs.append(osub)
              base = 0
              for si, nst in enumerate(SUBS):
                  nid = nst * STW
                  s = 0
                  while s < nid:
                      n = min(GCH, nid - s)
                      for (buf, idxs_t) in ((TsTs[si], sidx), (ToTs[si], oidx)):
                          nc.gpsimd.dma_gather(
                              out_ap=buf[:, :, s:s + n], in_ap=T_full[:],
                              idxs_ap=idxs_t[:, (base + s) // 16:(base + s + n) // 16],
                              num_idxs=n, num_idxs_reg=n, elem_size=P,
                              transpose=True, queue_num=0)
                      s += n
                  base += nid
            if stop_after == "gather":
                nc.sync.dma_start(out=dbg_d[:, 0:2048],
                                  in_=TsTs[0][:, 0, 0:2048])
            if stop_after == "gatherw":
                nc.sync.dma_start(out=dbg_d[:, 0:3328],
                                  in_=W_all[:].rearrange("p a b -> p (a b)"))

            if on("full"):
              scores = cp.tile([1, NST * STW], F16)
              st = 0
              for si, nst in enumerate(SUBS):
                for li in range(nst):
                 pY = psY.tile([P, STW], F32, tag="py")
                 nc.tensor.matmul(out=pY[:], lhsT=W_all[:, st, :],
                                  rhs=TsTs[si][:, 0, li * STW:(li + 1) * STW],
                                  start=True, stop=True)
                 prod = pb.tile([P, STW], F16, tag="prod")
                 nc.vector.tensor_tensor(
                     out=prod[:], in0=pY[:],
                     in1=ToTs[si][:, 0, li * STW:(li + 1) * STW], op=ALU.mult)
                 pS = psS.tile([1, STW], F32, tag="ps")
                 nc.tensor.matmul(out=pS[:], lhsT=ones[:], rhs=prod[:],
                                  start=True, stop=True)
                 nc.scalar.activation(
                     out=scores[0:1, st * STW:(st + 1) * STW], in_=pS[:],
                     func=ACTFN.Copy)
                 st += 1
              nc.sync.dma_start(out=scores_d[:], in_=scores[:])

    nc.finalize()
    return nc


# --------------------------------------------------------------------------
# host side
# --------------------------------------------------------------------------

def _rot_elems(params):
    """params [..., 3] (phi, theta, omega) -> v0..v3 f32.

    m00=(v0,-v1) m01=(-v2,-v3) m10=(v2,-v3) m11=(v0,v1)
    """
    phi, tha, omg = params[..., 0], params[..., 1], params[..., 2]
    ch, sh = np.cos(tha / 2), np.sin(tha / 2)
    a, b = (phi + omg) / 2, (phi - omg) / 2
    return (
        (ch * np.cos(a)).astype(np.float32),
        (ch * np.sin(a)).astype(np.float32),
        (sh * np.cos(b)).astype(np.float32),
        (sh * np.sin(b)).astype(np.float32),
    )


def _pack_idxs(flat):
    """int array [n] (n % 16 == 0) -> [128, n/16] int16 (replicated x8)."""
    n = len(flat)
    blk = np.asarray(flat, np.int16).reshape(n // 16, 16).T
    return np.tile(blk, (8, 1))


def _host_prep(entity_params, relation_params, s_idx, p_idx, o_idx):
    ent = np.asarray(entity_params, dtype=np.float32)
    rel = np.asarray(relation_params, dtype=np.float32)
    s_idx = np.asarray(s_idx)
    p_idx = np.asarray(p_idx)
    o_idx = np.asarray(o_idx)

    # ---- entity shards ----
    ent_flat = ent.reshape(E, 72)
    ent_pad = np.zeros((EPAD, 72), np.float32)
    ent_pad[:E] = ent_flat
    ent_shards = [
        ent_pad[c * EPC:(c + 1) * EPC].reshape(ETILES, P, 72)
        for c in range(NCORES)
    ]

    # ---- supertiles (relation, <=512 elements) ----
    order = np.argsort(p_idx, kind="stable")
    bounds = np.searchsorted(p_idx[order], np.arange(R + 1))
    tiles = []
    for r in range(R):
        grp = order[bounds[r]:bounds[r + 1]]
        for i in range(0, len(grp), STW):
            tiles.append((r, grp[i:i + STW]))
    ntiles = len(tiles)
    assert ntiles <= NCORES * NST, f"too many supertiles {ntiles}"

    # greedy balance: relations (grouped) to cores, bounded slots+tiles
    by_rel = {}
    for t in tiles:
        by_rel.setdefault(t[0], []).append(t)
    core_tiles = [[] for _ in range(NCORES)]
    for r, ts in sorted(by_rel.items(), key=lambda kv: -len(kv[1])):
        remaining = list(ts)
        while remaining:
            c = min(range(NCORES), key=lambda cc: len(core_tiles[cc]))
            cap = NST - len(core_tiles[c])
            assert cap > 0, "balance failure"
            take = remaining[:cap]
            core_tiles[c].extend(take)
            remaining = remaining[len(take):]

    gate_rel = rel.reshape(R, 24, 3)
    v0, v1, v2, v3 = _rot_elems(gate_rel)   # each [R, 24]

    in_maps = []
    outpos = np.full((NCORES, NST, STW), -1, np.int64)
    for c in range(NCORES):
        ct = core_tiles[c]
        assert len(ct) <= NST
        # static schedule: slot st computes supertile st's relation
        slot_rel = [ct[t][0] if t < len(ct) else 0 for t in range(RSLOT)]

        sflat = np.zeros(NIDX_T, np.int64)
        oflat = np.zeros(NIDX_T, np.int64)
        for t in range(NST):
            if t < len(ct):
                r, elems = ct[t]
                n = len(elems)
                sflat[t * STW:t * STW + n] = s_idx[elems]
                oflat[t * STW:t * STW + n] = o_idx[elems]
                outpos[c, t, :n] = elems

        # W-chain coef [p, 18, 7, WT] fp16 + doubling factors [p,6,6,WT]
        wcoefb = np.zeros((P, 18, 7, WT), np.float16)
        wfac = np.zeros((P, 6, 6, WT), np.float16)
        for sl, r in enumerate(slot_rel):
            wt, hh = divmod(sl, 2)
            rows = slice(64 * hh, 64 * hh + 64)
            for g in range(18):
                gg = 6 + g
                vals = (v0[r, gg], v1[r, gg], v2[r, gg], v3[r, gg],
                        -v1[r, gg], -v2[r, gg], -v3[r, gg])
                for k, vvv in enumerate(vals):
                    wcoefb[rows, g, k, wt] = vvv
            j = np.arange(64)
            for k in range(6):
                qg = 5 - k
                bit = (j >> k) & 1
                m00 = (v0[r, qg], -v1[r, qg])
                m01 = (-v2[r, qg], -v3[r, qg])
                m10 = (v2[r, qg], -v3[r, qg])
                m11 = (v0[r, qg], v1[r, qg])
                u0r = np.where(bit == 0, m00[0], m01[0])
                u0i = np.where(bit == 0, m00[1], m01[1])
                u1r = np.where(bit == 0, m10[0], m11[0])
                u1i = np.where(bit == 0, m10[1], m11[1])
                wfac[rows, k, 0, wt] = u0r
                wfac[rows, k, 1, wt] = u0i
                wfac[rows, k, 2, wt] = -u0i
                wfac[rows, k, 3, wt] = u1r
                wfac[rows, k, 4, wt] = u1i
                wfac[rows, k, 5, wt] = -u1i

        in_maps.append({
            "ent_par": ent_shards[c],
            "wcoefb": wcoefb,
            "wfac": wfac,
            "sidx": _pack_idxs(sflat),
            "oidx": _pack_idxs(oflat),
        })
    return in_maps, outpos


_PROGRAM = None


def kernel(entity_params, relation_params, s_idx, p_idx, o_idx):
    global _PROGRAM
    in_maps, outpos = _host_prep(entity_params, relation_params,
                                 s_idx, p_idx, o_idx)
    if _PROGRAM is None:
        _PROGRAM = build_program()
    nc = _PROGRAM
    res = run_bass_kernel_spmd(nc, in_maps, list(range(NCORES)))
    out = np.zeros(B, np.float32)
    for c in range(NCORES):
        sc = res.results[c]["scores"].astype(np.float32).reshape(NST, STW)
        pos = outpos[c]
        mask = pos >= 0
        out[pos[mask]] = sc[mask]
    return out


if __name__ == "__main__":
    build_program()
    print("build OK")
